# revision 50
# baseline (speedup 1.0000x reference)
"""Multi-head causal self-attention (B=1, S=4096, D=1024, H=16) on 8 TRN2 cores.

Sharding: 2 heads per core (head/tensor parallel). Each core computes its
heads' Q/K/V projections, causal flash attention, and a partial output
projection against its 128 columns of Wo. The host sums the 8 partials and
adds the output bias.

Device layouts (per core, bf16 compute):
  - x is fed transposed:  xT [D=1024, S=4096]   (model dim on partitions)
  - Q^T, K^T [128, 4096]: per-core head dims on partitions (h0: 0-63, h1: 64-127)
  - V natural [4096, 130]: per seq-tile [128, 65*2] = [V_h0 | ones | V_h1 | ones]
    The ones column makes the PV matmul also produce the softmax denominator.
  - scores are computed transposed S^T[k, q] so the PV matmul needs no
    transposition; softmax is exp-only (scores are bounded, no max-subtract).
  - output is written transposed outT [1024, 4096] bf16 (partial; host sums
    the 8 partials in fp32 — costs ~3e-4 relative error, well inside budget).
"""

import numpy as np
import ml_dtypes
from contextlib import ExitStack

import concourse.bass as bass
import concourse.tile as tile
from concourse import bacc, mybir
from concourse.bass_utils import run_bass_kernel_spmd

P = 128
S = 4096
D = 1024
DH = 64
N_CORES = 8
SCALE = 1.0 / 8.0  # 1/sqrt(64)
NQ = 512           # query block (matmul free dim)
KT = 128           # key tile (contraction partitions)
NQB = S // NQ      # 8 query blocks
NKT = S // KT      # 32 key tiles
KO = D // P        # 8 contraction subtiles over the model dim

BF16 = mybir.dt.bfloat16
F32 = mybir.dt.float32
EXP = mybir.ActivationFunctionType.Exp
ADD = mybir.AluOpType.add


def _emit(tc, xT, wqT, wkT, wvT, woT, bqk, bvrep, masks, outT, dbg=None):
    nc = tc.nc
    with ExitStack() as ctx:
        from collections import deque
        from concourse.masks import make_identity

        const = ctx.enter_context(tc.tile_pool(name="const", bufs=1))

        # weights/constants first: the first projections need wq + x chunk 0,
        # so nothing small may queue behind the 8.4MB xT transfer. All inputs
        # are host-prepacked to partition-contiguous layouts (128 descriptors
        # per transfer instead of 1024).
        wq_sb = const.tile([P, KO, P], BF16)
        nc.sync.dma_start(wq_sb, wqT)
        bqk_sb = const.tile([P, 3], F32)
        nc.sync.dma_start(bqk_sb, bqk.rearrange("b p -> p b"))
        xT_sb = const.tile([P, NQB, KO, NQ], BF16)
        nc.sync.dma_start(xT_sb[:, 0], xT[:, 0])  # q-proj of block 0 can start
        wk_sb = const.tile([P, KO, P], BF16)
        nc.sync.dma_start(wk_sb, wkT)
        wv_sb = const.tile([P, KO, P], BF16)
        nc.sync.dma_start(wv_sb, wvT)
        masks_sb = const.tile([P, KT], BF16)
        nc.sync.dma_start(masks_sb, masks)
        bv_sb = const.tile([P, P], F32)
        nc.sync.dma_start(bv_sb, bvrep)
        nc.sync.dma_start(xT_sb[:, 1], xT[:, 1])
        wo_sb = const.tile([P, D], BF16)
        nc.sync.dma_start(wo_sb, woT)
        for n in range(2, NQB):  # chunked so projections can start early
            nc.sync.dma_start(xT_sb[:, n], xT[:, n])

        qT_sb = const.tile([P, S], BF16)
        kT_sb = const.tile([P, S], BF16)
        v_sb = const.tile([P, S // P, 130], BF16)
        attnT_sb = const.tile([P, S], BF16)
        nc.vector.memset(v_sb, 1.0)  # presets the two ones-columns

        ident = const.tile([P, P], BF16)
        make_identity(nc, ident)

        # Warm the PE clock (HAM) with throwaway matmuls while the input DMAs
        # land. The HAM needs ~3.4us of *sustained* PE activity to unthrottle
        # (cold MMs run at 1.2GHz, so 60 x N=128 ~ 6.4us) — and the burst must
        # also bridge the DMA wait so the first projections start warm.
        with tc.tile_pool(name="warm_psum", bufs=1, space="PSUM") as wpool:
            wt = wpool.tile([P, P], F32)
            for _ in range(60):
                nc.tensor.matmul(wt, lhsT=ident, rhs=ident, start=True, stop=True)

        # PSUM budget (8 banks): spool 4 (two [128,2,512] score slabs),
        # vpool 2 (pv0/pv1 accumulators), ppool 1 (proj accum / V transpose),
        # opool 1 (output projection).
        spool = ctx.enter_context(tc.tile_pool(name="score_psum", bufs=2, space="PSUM"))
        vpool = ctx.enter_context(tc.tile_pool(name="pv_psum", bufs=1, space="PSUM"))
        ppool = ctx.enter_context(tc.tile_pool(name="proj_psum", bufs=1, space="PSUM"))
        opool = ctx.enter_context(tc.tile_pool(name="oproj_psum", bufs=1, space="PSUM"))
        work = ctx.enter_context(tc.tile_pool(name="work", bufs=7))
        nwork = ctx.enter_context(tc.tile_pool(name="nwork", bufs=3))
        dpool = ctx.enter_context(tc.tile_pool(name="dscratch", bufs=2, space="DRAM"))

        def proj_chunk(bcol, w_sb, dst, n):
            """Two pacing items of 4 accumulation matmuls each (shared psum)."""
            state = {}

            def emit_lo():
                ps = ppool.tile([P, NQ], F32, tag="ps", name=f"ps_{bcol}_{n}")
                state["ps"] = ps
                for kt in range(KO // 2):
                    nc.tensor.matmul(
                        ps,
                        lhsT=w_sb[:, kt, :],
                        rhs=xT_sb[:, n, kt, :],
                        start=(kt == 0),
                        stop=False,
                    )

            def emit_hi():
                ps = state["ps"]
                for kt in range(KO // 2, KO):
                    nc.tensor.matmul(
                        ps,
                        lhsT=w_sb[:, kt, :],
                        rhs=xT_sb[:, n, kt, :],
                        start=False,
                        stop=(kt == KO - 1),
                    )
                nc.vector.tensor_tensor(
                    dst[:, n * NQ:(n + 1) * NQ],
                    ps,
                    bqk_sb[:, bcol:bcol + 1].to_broadcast([P, NQ]),
                    op=ADD,
                )

            return [emit_lo, emit_hi]

        def v_proj(t):
            """V in natural [seq, dh] layout: stationary operand is the xT
            chunk, so no PE transpose is needed afterwards. One seq-tile of
            128 rows lands as [128, 2, 64] inside v_sb (ones columns preset)."""
            def emit():
                ps = ppool.tile([P, P], F32, tag="ps", name=f"vp_{t}")
                for kt in range(KO):
                    nc.tensor.matmul(
                        ps,
                        lhsT=xT_sb[:, t // 4, kt, (t % 4) * P:(t % 4 + 1) * P],
                        rhs=wv_sb[:, kt, :],
                        start=(kt == 0),
                        stop=(kt == KO - 1),
                    )
                nc.vector.tensor_tensor(
                    v_sb[:, t, :].rearrange("p (h x) -> p h x", x=65)[:, :, 0:DH],
                    ps.rearrange("p (h x) -> p h x", x=DH),
                    bv_sb.rearrange("p (h x) -> p h x", x=DH),
                    op=ADD,
                )
            return emit

        def proj_ops(nb):
            ops = []
            ops += proj_chunk(0, wq_sb, qT_sb, nb)
            ops += proj_chunk(1, wk_sb, kT_sb, nb)
            ops += [v_proj(t) for t in range(4 * nb, 4 * nb + 4)]
            return ops

        def oproj_mtile(b, m, tail=False):
            def emit():
                qsl = slice(b * NQ, (b + 1) * NQ)
                # in the drain after the last block, alternate PSUM banks and
                # cast engines so consecutive m-tiles don't serialize on the
                # single oproj bank / the DVE
                use_p = tail and (m % 2 == 1)
                pool, tag = (ppool, "ps") if use_p else (opool, "po")
                po = pool.tile([P, NQ], F32, tag=tag, name=f"po_{b}_{m}")
                nc.tensor.matmul(
                    po,
                    lhsT=wo_sb[:, m * P:(m + 1) * P],
                    rhs=attnT_sb[:, qsl],
                    start=True,
                    stop=True,
                )
                ot = work.tile([P, NQ], BF16, tag="ot", name=f"ot_{b}_{m}")
                if use_p:
                    nc.scalar.copy(ot, po)
                else:
                    nc.vector.tensor_copy(ot, po)
                nc.sync.dma_start(
                    outT.rearrange("(mo p) n -> p mo n", p=P)[:, m, qsl], ot
                )
            return emit

        # block 0's q/k projections up front; its v tiles are only needed by
        # the PV matmuls and would delay the first scores/EXP, so they go
        # into block 0's paced queue instead.
        for op in proj_chunk(0, wq_sb, qT_sb, 0) + proj_chunk(1, wk_sb, kT_sb, 0):
            op()

        ones1 = const.tile([1, DH], F32)
        nc.vector.memset(ones1, 1.0)

        def make_norm(b, pvs, tail=False):
            """Normalize block b's PV accumulators into attnT. Heads are
            interleaved so the DVE works on head 1 while head 0's reciprocal
            broadcast makes its DRAM round-trip. In the tail (last block) the
            broadcast is a K=1 PE matmul instead — the PE is idle there and
            it avoids ~2us of DMA latency on the critical path."""
            def norm():
                qsl = slice(b * NQ, (b + 1) * NQ)
                pvS_t, rb_t = [], []
                for h in (0, 1):
                    if tail:
                        # denominator row first: the reciprocal + broadcast
                        # then overlap the bulk PSUM->SBUF copy
                        rcp0 = nwork.tile([1, NQ], F32, tag=f"rcp0{h}")
                        nc.vector.tensor_copy(rcp0, pvs[h][DH:DH + 1, :])
                        nc.vector.reciprocal_approx_fast(rcp0, rcp0)
                        pvS = nwork.tile([DH + 1, NQ], F32, tag=f"pvS{h}")
                        nc.vector.tensor_copy(pvS, pvs[h])
                    else:
                        # PSUM-freeing copy first: the next block's first PV
                        # matmul reuses this bank and must not wait
                        pvS = nwork.tile([DH + 1, NQ], F32, tag=f"pvS{h}")
                        nc.vector.tensor_copy(pvS, pvs[h])
                        rcp0 = nwork.tile([1, NQ], F32, tag=f"rcp0{h}")
                        nc.vector.tensor_copy(rcp0, pvS[DH:DH + 1, :])
                        nc.vector.reciprocal_approx_fast(rcp0, rcp0)
                    if tail:
                        rb = vpool.tile([DH + 1, NQ], F32, tag=f"pv{h}",
                                        name=f"rb_{b}_{h}")
                        nc.tensor.matmul(rb[0:DH, :], lhsT=ones1, rhs=rcp0,
                                         start=True, stop=True)
                    else:
                        scr = dpool.tile([NQ], F32, tag=f"scr{h}")
                        nc.sync.dma_start(scr, rcp0)
                        rb = nwork.tile([DH, NQ], F32, tag=f"rb{h}")
                        nc.sync.dma_start(rb, scr[None, :].to_broadcast([DH, NQ]))
                    pvS_t.append(pvS)
                    rb_t.append(rb)
                for h in (0, 1):
                    nc.vector.tensor_mul(attnT_sb[h * DH:(h + 1) * DH, qsl],
                                         pvS_t[h][0:DH, :], rb_t[h][0:DH, :])
                    if dbg is not None and b == 1:
                        nc.sync.dma_start(dbg[f"pv{h}"], pvS_t[h])
            return norm

        soft_q = deque()  # oproj items: no hard deadline, trickled 1 per kt
        prev = None       # (emit_pv, state): PV runs one k-tile behind, and
                          # rolls across block boundaries to keep the PE fed
        norm_fn = None    # previous block's normalization, emitted after the
                          # next block's first scores/exp so ACT never waits
        for b in range(NQB):
            nk = 4 * (b + 1)  # causal: only key tiles up to the diagonal
            # q/k/v projections of block b+1 must finish during block b; give
            # each item an even deadline so there is no burst at the boundary.
            hard_q = deque(proj_ops(b + 1)) if b + 1 < NQB else deque()
            if b == 0:
                hard_q.extendleft(reversed([v_proj(t) for t in range(4)]))
            nhard = len(hard_q)
            if b > 0:
                soft_q.extend(oproj_mtile(b - 1, m) for m in range(D // P))
            pvs = [
                vpool.tile([DH + 1, NQ], F32, tag=f"pv{h}", name=f"pv{h}_{b}")
                for h in (0, 1)
            ]

            def emit_pv(st, pvs=pvs, nk=nk):
                pT, kt, q0, nq = st
                for h in (0, 1):
                    nc.tensor.matmul(
                        pvs[h][:, q0:],
                        lhsT=v_sb[:, kt, h * 65:(h + 1) * 65],
                        rhs=pT[:, h, :nq],
                        start=(kt == 0),
                        stop=(kt == nk - 1),
                    )

            for kt in range(nk):
                j = kt - 4 * b  # >= 0 on causal-diagonal key tiles
                # on diagonal tiles only queries >= 128j can attend this tile
                q0 = max(0, j) * KT
                nq = NQ - q0
                qs0 = b * NQ + q0
                # mid-block: PV of the previous tile goes first (it is ready —
                # scores may still wait on its slab). At a block boundary the
                # rolled-over PV waits on the last diagonal mask, so scores
                # go first to keep ACT fed across the boundary.
                if prev is not None and kt > 0:
                    prev[0](prev[1])
                    prev = None
                slab = spool.tile([P, 2, NQ], F32, tag="slab")
                for h in (0, 1):
                    nc.tensor.matmul(
                        slab[:, h, :nq],
                        lhsT=kT_sb[h * DH:(h + 1) * DH, kt * KT:(kt + 1) * KT],
                        rhs=qT_sb[h * DH:(h + 1) * DH, qs0:qs0 + nq],
                        start=True,
                        stop=True,
                    )
                if prev is not None:
                    prev[0](prev[1])
                    prev = None
                pT = work.tile([P, 2, NQ], BF16, tag="pT")
                nc.scalar.activation(pT[:, :, :nq], slab[:, :, :nq], EXP, scale=SCALE)
                if j >= 0:
                    # causal mask: within a diagonal tile only the first 128
                    # queries (qq < 128 relative to q0) form a triangle with
                    # the 128 keys; later queries see the whole tile.
                    nc.vector.tensor_mul(
                        pT[:, :, 0:KT],
                        pT[:, :, 0:KT],
                        masks_sb.rearrange("p (o q) -> p o q", o=1).to_broadcast(
                            [P, 2, KT]
                        ),
                    )
                if norm_fn is not None:
                    norm_fn()
                    norm_fn = None
                prev = (emit_pv, (pT, kt, q0, nq))
                # hard items: evenly spread, finishing ~2 tiles before the
                # boundary so the last bias-add isn't racing the next block's
                # first scores on the DVE
                done_hard = nhard - len(hard_q)
                due = -(-nhard * (kt + 1) // (nk - 2 if nk >= 8 else nk))
                while hard_q and done_hard < due:
                    hard_q.popleft()()
                    done_hard += 1
                # at most one oproj per key tile (single-buffered PSUM bank),
                # and only once the previous block's attnT normalization has
                # had time to finish — else its first matmul blocks the PE FIFO
                if soft_q and kt >= 4:
                    soft_q.popleft()()
            while hard_q:
                hard_q.popleft()()
            norm_fn = make_norm(b, pvs, tail=(b == NQB - 1))
        prev[0](prev[1])
        norm_fn()
        while soft_q:
            soft_q.popleft()()
        for m in range(D // P):
            oproj_mtile(NQB - 1, m, tail=True)()
        if dbg is not None:
            nc.sync.dma_start(dbg["qT"], qT_sb)
            nc.sync.dma_start(dbg["kT"], kT_sb)
            nc.sync.dma_start(dbg["v"], v_sb)
            nc.sync.dma_start(dbg["attnT"], attnT_sb)


def build(debug_out=False):
    nc = bacc.Bacc(
        "TRN2",
        target_bir_lowering=False,
        debug=False,
        enable_asserts=False,
    )
    xT = nc.dram_tensor("xT", [P, NQB, KO, NQ], BF16, kind="ExternalInput").ap()
    wqT = nc.dram_tensor("wqT", [P, KO, P], BF16, kind="ExternalInput").ap()
    wkT = nc.dram_tensor("wkT", [P, KO, P], BF16, kind="ExternalInput").ap()
    wvT = nc.dram_tensor("wvT", [P, KO, P], BF16, kind="ExternalInput").ap()
    woT = nc.dram_tensor("woT", [P, D], BF16, kind="ExternalInput").ap()
    bqk = nc.dram_tensor("bqk", [3, P], F32, kind="ExternalInput").ap()
    bvrep = nc.dram_tensor("bvrep", [P, P], F32, kind="ExternalInput").ap()
    masks = nc.dram_tensor("masks", [P, KT], BF16, kind="ExternalInput").ap()
    outT = nc.dram_tensor("outT", [D, S], BF16, kind="ExternalOutput").ap()
    dbg = None
    if debug_out:
        dbg = {
            "qT": nc.dram_tensor("dbg_qT", [P, S], BF16, kind="ExternalOutput").ap(),
            "kT": nc.dram_tensor("dbg_kT", [P, S], BF16, kind="ExternalOutput").ap(),
            "v": nc.dram_tensor("dbg_v", [P, S // P, 130], BF16, kind="ExternalOutput").ap(),
            "attnT": nc.dram_tensor("dbg_attnT", [P, S], BF16, kind="ExternalOutput").ap(),
            "pv0": nc.dram_tensor("dbg_pv0", [DH + 1, NQ], F32, kind="ExternalOutput").ap(),
            "pv1": nc.dram_tensor("dbg_pv1", [DH + 1, NQ], F32, kind="ExternalOutput").ap(),
        }

    with tile.TileContext(nc) as tc:
        _emit(tc, xT, wqT, wkT, wvT, woT, bqk, bvrep, masks, outT, dbg=dbg)
    nc.compile()
    return nc


def _make_masks():
    k = np.arange(P)[:, None]
    q = np.arange(KT)[None, :]
    return (k <= q).astype(ml_dtypes.bfloat16)


_STATE = {}


def _prep_inputs(x, Wq, bq, Wk, bk, Wv, bv, Wo, bo):
    bf = ml_dtypes.bfloat16
    x2 = np.asarray(x, np.float32).reshape(NQB, NQ, KO, P)
    # xT[p, nb, ko, nq] = x[nb*NQ+nq, ko*P+p]: partition-contiguous chunks
    xT = np.ascontiguousarray(x2.transpose(3, 0, 2, 1)).astype(bf)
    masks = _make_masks()
    Wq = np.asarray(Wq, np.float32)
    Wk = np.asarray(Wk, np.float32)
    Wv = np.asarray(Wv, np.float32)
    Wo = np.asarray(Wo, np.float32)
    bq = np.asarray(bq, np.float32)
    bk = np.asarray(bk, np.float32)
    bv = np.asarray(bv, np.float32)

    def wpack(w_rows):  # [128(m), D] -> [p, ko, m]
        return np.ascontiguousarray(
            w_rows.reshape(P, KO, P).transpose(2, 1, 0)
        ).astype(bf)

    in_maps = []
    for c in range(N_CORES):
        r = slice(c * P, (c + 1) * P)
        in_maps.append({
            "xT": xT,
            "wqT": wpack(Wq[r]),
            "wkT": wpack(Wk[r]),
            "wvT": wpack(Wv[r]),
            "woT": np.ascontiguousarray(Wo[:, r].T).astype(bf),
            "bqk": np.stack([bq[r], bk[r], bv[r]]),
            "bvrep": np.tile(bv[r][None, :], (P, 1)).astype(np.float32),
            "masks": masks,
        })
    return in_maps


def kernel(x, Wq, bq, Wk, bk, Wv, bv, Wo, bo):
    if "nc" not in _STATE:
        _STATE["nc"] = build()
    nc = _STATE["nc"]
    in_maps = _prep_inputs(x, Wq, bq, Wk, bk, Wv, bv, Wo, bo)
    res = run_bass_kernel_spmd(nc, in_maps, core_ids=list(range(N_CORES)))
    total = res.results[0]["outT"].astype(np.float32, copy=True)
    for c in range(1, N_CORES):
        total += res.results[c]["outT"]
    out = total.T + np.asarray(bo, np.float32)[None, :]
    return np.ascontiguousarray(out, dtype=np.float32).reshape(1, S, D)



# revision 51
# speedup vs baseline: 1.0238x; 1.0238x over previous
"""Multi-head causal self-attention (B=1, S=4096, D=1024, H=16) on 8 TRN2 cores.

Sharding: 2 heads per core (head/tensor parallel). Each core computes its
heads' Q/K/V projections, causal flash attention, and a partial output
projection against its 128 columns of Wo. The host sums the 8 partials and
adds the output bias.

Device layouts (per core, bf16 compute):
  - x is fed transposed:  xT [D=1024, S=4096]   (model dim on partitions)
  - Q^T, K^T [128, 4096]: per-core head dims on partitions (h0: 0-63, h1: 64-127)
  - V natural [4096, 130]: per seq-tile [128, 65*2] = [V_h0 | ones | V_h1 | ones]
    The ones column makes the PV matmul also produce the softmax denominator.
  - scores are computed transposed S^T[k, q] so the PV matmul needs no
    transposition; softmax is exp-only (scores are bounded, no max-subtract).
  - output is written transposed outT [1024, 4096] bf16 (partial; host sums
    the 8 partials in fp32 — costs ~3e-4 relative error, well inside budget).
"""

import numpy as np
import ml_dtypes
from contextlib import ExitStack

import concourse.bass as bass
import concourse.tile as tile
from concourse import bacc, mybir
from concourse.bass_utils import run_bass_kernel_spmd

P = 128
S = 4096
D = 1024
DH = 64
N_CORES = 8
SCALE = 1.0 / 8.0  # 1/sqrt(64)
NQ = 512           # query block (matmul free dim)
KT = 128           # key tile (contraction partitions)
NQB = S // NQ      # 8 query blocks
NKT = S // KT      # 32 key tiles
KO = D // P        # 8 contraction subtiles over the model dim

BF16 = mybir.dt.bfloat16
F32 = mybir.dt.float32
EXP = mybir.ActivationFunctionType.Exp
ADD = mybir.AluOpType.add


def _emit(tc, xT, wqT, wkT, wvT, woT, bqk, bvrep, masks, outT, dbg=None):
    nc = tc.nc
    with ExitStack() as ctx:
        from collections import deque
        from concourse.masks import make_identity

        const = ctx.enter_context(tc.tile_pool(name="const", bufs=1))

        # weights/constants first: the first projections need wq + x chunk 0,
        # so nothing small may queue behind the 8.4MB xT transfer. All inputs
        # are host-prepacked to partition-contiguous layouts (128 descriptors
        # per transfer instead of 1024).
        wq_sb = const.tile([P, KO, P], BF16)
        nc.sync.dma_start(wq_sb, wqT)
        bqk_sb = const.tile([P, 3], F32)
        nc.sync.dma_start(bqk_sb, bqk.rearrange("b p -> p b"))
        xT_sb = const.tile([P, NQB, KO, NQ], BF16)
        nc.sync.dma_start(xT_sb[:, 0], xT[:, 0])  # q-proj of block 0 can start
        wk_sb = const.tile([P, KO, P], BF16)
        nc.sync.dma_start(wk_sb, wkT)
        wv_sb = const.tile([P, KO, P], BF16)
        nc.sync.dma_start(wv_sb, wvT)
        masks_sb = const.tile([P, KT], BF16)
        nc.sync.dma_start(masks_sb, masks)
        bv_sb = const.tile([P, P], F32)
        nc.sync.dma_start(bv_sb, bvrep)
        nc.sync.dma_start(xT_sb[:, 1], xT[:, 1])
        wo_sb = const.tile([P, D], BF16)
        nc.sync.dma_start(wo_sb, woT)
        for n in range(2, NQB):  # chunked so projections can start early
            nc.sync.dma_start(xT_sb[:, n], xT[:, n])

        qT_sb = const.tile([P, S], BF16)
        kT_sb = const.tile([P, S], BF16)
        v_sb = const.tile([P, S // P, 130], BF16)
        attnT_sb = const.tile([P, S], BF16)
        nc.vector.memset(v_sb, 1.0)  # presets the two ones-columns

        ident = const.tile([P, P], BF16)
        make_identity(nc, ident)

        # Warm the PE clock (HAM) with throwaway matmuls while the input DMAs
        # land. The HAM needs ~3.4us of *sustained* PE activity to unthrottle
        # (cold MMs run at 1.2GHz, so 60 x N=128 ~ 6.4us) — and the burst must
        # also bridge the DMA wait so the first projections start warm.
        with tc.tile_pool(name="warm_psum", bufs=1, space="PSUM") as wpool:
            wt = wpool.tile([P, P], F32)
            for _ in range(60):
                nc.tensor.matmul(wt, lhsT=ident, rhs=ident, start=True, stop=True)

        # PSUM budget (8 banks): spool 4 (two [128,2,512] score slabs),
        # vpool 2 (pv0/pv1 accumulators), ppool 1 (proj accum / V transpose),
        # opool 1 (output projection).
        spool = ctx.enter_context(tc.tile_pool(name="score_psum", bufs=2, space="PSUM"))
        vpool = ctx.enter_context(tc.tile_pool(name="pv_psum", bufs=1, space="PSUM"))
        ppool = ctx.enter_context(tc.tile_pool(name="proj_psum", bufs=1, space="PSUM"))
        opool = ctx.enter_context(tc.tile_pool(name="oproj_psum", bufs=1, space="PSUM"))
        work = ctx.enter_context(tc.tile_pool(name="work", bufs=7))
        nwork = ctx.enter_context(tc.tile_pool(name="nwork", bufs=3))
        dpool = ctx.enter_context(tc.tile_pool(name="dscratch", bufs=2, space="DRAM"))

        def proj_chunk(bcol, w_sb, dst, n):
            """Two pacing items of 4 accumulation matmuls each (shared psum)."""
            state = {}

            def emit_lo():
                ps = ppool.tile([P, NQ], F32, tag="ps", name=f"ps_{bcol}_{n}")
                state["ps"] = ps
                for kt in range(KO // 2):
                    nc.tensor.matmul(
                        ps,
                        lhsT=w_sb[:, kt, :],
                        rhs=xT_sb[:, n, kt, :],
                        start=(kt == 0),
                        stop=False,
                    )

            def emit_hi():
                ps = state["ps"]
                for kt in range(KO // 2, KO):
                    nc.tensor.matmul(
                        ps,
                        lhsT=w_sb[:, kt, :],
                        rhs=xT_sb[:, n, kt, :],
                        start=False,
                        stop=(kt == KO - 1),
                    )
                nc.vector.tensor_tensor(
                    dst[:, n * NQ:(n + 1) * NQ],
                    ps,
                    bqk_sb[:, bcol:bcol + 1].to_broadcast([P, NQ]),
                    op=ADD,
                )

            return [emit_lo, emit_hi]

        def v_proj(t):
            """V in natural [seq, dh] layout: stationary operand is the xT
            chunk, so no PE transpose is needed afterwards. One seq-tile of
            128 rows lands as [128, 2, 64] inside v_sb (ones columns preset)."""
            def emit():
                ps = ppool.tile([P, P], F32, tag="ps", name=f"vp_{t}")
                for kt in range(KO):
                    nc.tensor.matmul(
                        ps,
                        lhsT=xT_sb[:, t // 4, kt, (t % 4) * P:(t % 4 + 1) * P],
                        rhs=wv_sb[:, kt, :],
                        start=(kt == 0),
                        stop=(kt == KO - 1),
                    )
                nc.vector.tensor_tensor(
                    v_sb[:, t, :].rearrange("p (h x) -> p h x", x=65)[:, :, 0:DH],
                    ps.rearrange("p (h x) -> p h x", x=DH),
                    bv_sb.rearrange("p (h x) -> p h x", x=DH),
                    op=ADD,
                )
            return emit

        def proj_ops(nb):
            ops = []
            ops += proj_chunk(0, wq_sb, qT_sb, nb)
            ops += proj_chunk(1, wk_sb, kT_sb, nb)
            ops += [v_proj(t) for t in range(4 * nb, 4 * nb + 4)]
            return ops

        def oproj_mtile(b, m, tail=False):
            def emit():
                qsl = slice(b * NQ, (b + 1) * NQ)
                # in the drain after the last block, alternate PSUM banks and
                # cast engines so consecutive m-tiles don't serialize on the
                # single oproj bank / the DVE
                use_p = tail and (m % 2 == 1)
                pool, tag = (ppool, "ps") if use_p else (opool, "po")
                po = pool.tile([P, NQ], F32, tag=tag, name=f"po_{b}_{m}")
                nc.tensor.matmul(
                    po,
                    lhsT=wo_sb[:, m * P:(m + 1) * P],
                    rhs=attnT_sb[:, qsl],
                    start=True,
                    stop=True,
                )
                ot = work.tile([P, NQ], BF16, tag="ot", name=f"ot_{b}_{m}")
                if use_p:
                    nc.scalar.copy(ot, po)
                else:
                    nc.vector.tensor_copy(ot, po)
                nc.sync.dma_start(
                    outT.rearrange("(mo p) n -> p mo n", p=P)[:, m, qsl], ot
                )
            return emit

        # block 0's q/k projections up front; its v tiles are only needed by
        # the PV matmuls and would delay the first scores/EXP, so they go
        # into block 0's paced queue instead.
        for op in proj_chunk(0, wq_sb, qT_sb, 0) + proj_chunk(1, wk_sb, kT_sb, 0):
            op()

        ones1 = const.tile([1, DH], F32)
        nc.vector.memset(ones1, 1.0)

        def make_norm(b, pvs, tail=False):
            """Normalize block b's PV accumulators into attnT. Heads are
            interleaved so the DVE works on head 1 while head 0's reciprocal
            broadcast makes its DRAM round-trip. In the tail (last block) the
            broadcast is a K=1 PE matmul instead — the PE is idle there and
            it avoids ~2us of DMA latency on the critical path."""
            def norm():
                qsl = slice(b * NQ, (b + 1) * NQ)
                pvS_t, rb_t = [], []
                for h in (0, 1):
                    if tail:
                        # denominator row first: the reciprocal + broadcast
                        # then overlap the bulk PSUM->SBUF copy
                        rcp0 = nwork.tile([1, NQ], F32, tag=f"rcp0{h}")
                        nc.vector.tensor_copy(rcp0, pvs[h][DH:DH + 1, :])
                        nc.vector.reciprocal_approx_fast(rcp0, rcp0)
                        pvS = nwork.tile([DH + 1, NQ], F32, tag=f"pvS{h}")
                        nc.vector.tensor_copy(pvS, pvs[h])
                    else:
                        # PSUM-freeing copy first: the next block's first PV
                        # matmul reuses this bank and must not wait
                        pvS = nwork.tile([DH + 1, NQ], F32, tag=f"pvS{h}")
                        nc.vector.tensor_copy(pvS, pvs[h])
                        rcp0 = nwork.tile([1, NQ], F32, tag=f"rcp0{h}")
                        nc.vector.tensor_copy(rcp0, pvS[DH:DH + 1, :])
                        nc.vector.reciprocal_approx_fast(rcp0, rcp0)
                    if tail:
                        rb = vpool.tile([DH + 1, NQ], F32, tag=f"pv{h}",
                                        name=f"rb_{b}_{h}")
                        nc.tensor.matmul(rb[0:DH, :], lhsT=ones1, rhs=rcp0,
                                         start=True, stop=True)
                    else:
                        scr = dpool.tile([NQ], F32, tag=f"scr{h}")
                        nc.sync.dma_start(scr, rcp0)
                        rb = nwork.tile([DH, NQ], F32, tag=f"rb{h}")
                        nc.sync.dma_start(rb, scr[None, :].to_broadcast([DH, NQ]))
                    pvS_t.append(pvS)
                    rb_t.append(rb)
                for h in (0, 1):
                    nc.vector.tensor_mul(attnT_sb[h * DH:(h + 1) * DH, qsl],
                                         pvS_t[h][0:DH, :], rb_t[h][0:DH, :])
                    if dbg is not None and b == 1:
                        nc.sync.dma_start(dbg[f"pv{h}"], pvS_t[h])
            return norm

        soft_q = deque()  # oproj items: no hard deadline, trickled 1 per kt
        prev = None       # (emit_pv, state): PV runs one k-tile behind, and
                          # rolls across block boundaries to keep the PE fed
        norm_fn = None    # previous block's normalization, emitted after the
                          # next block's first scores/exp so ACT never waits
        for b in range(NQB):
            nk = 4 * (b + 1)  # causal: only key tiles up to the diagonal
            # q/k/v projections of block b+1 must finish during block b; give
            # each item an even deadline so there is no burst at the boundary.
            hard_q = deque(proj_ops(b + 1)) if b + 1 < NQB else deque()
            if b == 0:
                hard_q.extendleft(reversed([v_proj(t) for t in range(4)]))
            nhard = len(hard_q)
            if b > 0:
                soft_q.extend(oproj_mtile(b - 1, m) for m in range(D // P))
            pvs = [
                vpool.tile([DH + 1, NQ], F32, tag=f"pv{h}", name=f"pv{h}_{b}")
                for h in (0, 1)
            ]

            def emit_pv(st, pvs=pvs, nk=nk):
                pT, kt, q0, nq = st
                for h in (0, 1):
                    nc.tensor.matmul(
                        pvs[h][:, q0:],
                        lhsT=v_sb[:, kt, h * 65:(h + 1) * 65],
                        rhs=pT[:, h, :nq],
                        start=(kt == 0),
                        stop=(kt == nk - 1),
                    )

            for kt in range(nk):
                j = kt - 4 * b  # >= 0 on causal-diagonal key tiles
                # on diagonal tiles only queries >= 128j can attend this tile
                q0 = max(0, j) * KT
                nq = NQ - q0
                qs0 = b * NQ + q0
                # mid-block: PV of the previous tile goes first (it is ready —
                # scores may still wait on its slab). At a block boundary the
                # rolled-over PV waits on the last diagonal mask, so scores
                # go first to keep ACT fed across the boundary.
                if prev is not None and kt > 0:
                    prev[0](prev[1])
                    prev = None
                slab = spool.tile([P, 2, NQ], F32, tag="slab")
                for h in (0, 1):
                    nc.tensor.matmul(
                        slab[:, h, :nq],
                        lhsT=kT_sb[h * DH:(h + 1) * DH, kt * KT:(kt + 1) * KT],
                        rhs=qT_sb[h * DH:(h + 1) * DH, qs0:qs0 + nq],
                        start=True,
                        stop=True,
                    )
                if prev is not None:
                    prev[0](prev[1])
                    prev = None
                pT = work.tile([P, 2, NQ], BF16, tag="pT")
                nc.scalar.activation(pT[:, :, :nq], slab[:, :, :nq], EXP, scale=SCALE)
                if j >= 0:
                    # causal mask: within a diagonal tile only the first 128
                    # queries (qq < 128 relative to q0) form a triangle with
                    # the 128 keys; later queries see the whole tile.
                    nc.vector.tensor_mul(
                        pT[:, :, 0:KT],
                        pT[:, :, 0:KT],
                        masks_sb.rearrange("p (o q) -> p o q", o=1).to_broadcast(
                            [P, 2, KT]
                        ),
                    )
                if norm_fn is not None:
                    norm_fn()
                    norm_fn = None
                prev = (emit_pv, (pT, kt, q0, nq))
                # hard items: evenly spread, finishing ~2 tiles before the
                # boundary so the last bias-add isn't racing the next block's
                # first scores on the DVE
                done_hard = nhard - len(hard_q)
                due = -(-nhard * (kt + 1) // max(1, nk - 2))
                while hard_q and done_hard < due:
                    hard_q.popleft()()
                    done_hard += 1
                # at most one oproj per key tile (single-buffered PSUM bank),
                # and only once the previous block's attnT normalization has
                # had time to finish — else its first matmul blocks the PE FIFO
                if soft_q and kt >= 4:
                    soft_q.popleft()()
            while hard_q:
                hard_q.popleft()()
            norm_fn = make_norm(b, pvs, tail=(b == NQB - 1))
        prev[0](prev[1])
        norm_fn()
        while soft_q:
            soft_q.popleft()()
        for m in range(D // P):
            oproj_mtile(NQB - 1, m, tail=True)()
        if dbg is not None:
            nc.sync.dma_start(dbg["qT"], qT_sb)
            nc.sync.dma_start(dbg["kT"], kT_sb)
            nc.sync.dma_start(dbg["v"], v_sb)
            nc.sync.dma_start(dbg["attnT"], attnT_sb)


def build(debug_out=False):
    nc = bacc.Bacc(
        "TRN2",
        target_bir_lowering=False,
        debug=False,
        enable_asserts=False,
    )
    xT = nc.dram_tensor("xT", [P, NQB, KO, NQ], BF16, kind="ExternalInput").ap()
    wqT = nc.dram_tensor("wqT", [P, KO, P], BF16, kind="ExternalInput").ap()
    wkT = nc.dram_tensor("wkT", [P, KO, P], BF16, kind="ExternalInput").ap()
    wvT = nc.dram_tensor("wvT", [P, KO, P], BF16, kind="ExternalInput").ap()
    woT = nc.dram_tensor("woT", [P, D], BF16, kind="ExternalInput").ap()
    bqk = nc.dram_tensor("bqk", [3, P], F32, kind="ExternalInput").ap()
    bvrep = nc.dram_tensor("bvrep", [P, P], F32, kind="ExternalInput").ap()
    masks = nc.dram_tensor("masks", [P, KT], BF16, kind="ExternalInput").ap()
    outT = nc.dram_tensor("outT", [D, S], BF16, kind="ExternalOutput").ap()
    dbg = None
    if debug_out:
        dbg = {
            "qT": nc.dram_tensor("dbg_qT", [P, S], BF16, kind="ExternalOutput").ap(),
            "kT": nc.dram_tensor("dbg_kT", [P, S], BF16, kind="ExternalOutput").ap(),
            "v": nc.dram_tensor("dbg_v", [P, S // P, 130], BF16, kind="ExternalOutput").ap(),
            "attnT": nc.dram_tensor("dbg_attnT", [P, S], BF16, kind="ExternalOutput").ap(),
            "pv0": nc.dram_tensor("dbg_pv0", [DH + 1, NQ], F32, kind="ExternalOutput").ap(),
            "pv1": nc.dram_tensor("dbg_pv1", [DH + 1, NQ], F32, kind="ExternalOutput").ap(),
        }

    with tile.TileContext(nc) as tc:
        _emit(tc, xT, wqT, wkT, wvT, woT, bqk, bvrep, masks, outT, dbg=dbg)
    nc.compile()
    return nc


def _make_masks():
    k = np.arange(P)[:, None]
    q = np.arange(KT)[None, :]
    return (k <= q).astype(ml_dtypes.bfloat16)


_STATE = {}


def _prep_inputs(x, Wq, bq, Wk, bk, Wv, bv, Wo, bo):
    bf = ml_dtypes.bfloat16
    x2 = np.asarray(x, np.float32).reshape(NQB, NQ, KO, P)
    # xT[p, nb, ko, nq] = x[nb*NQ+nq, ko*P+p]: partition-contiguous chunks
    xT = np.ascontiguousarray(x2.transpose(3, 0, 2, 1)).astype(bf)
    masks = _make_masks()
    Wq = np.asarray(Wq, np.float32)
    Wk = np.asarray(Wk, np.float32)
    Wv = np.asarray(Wv, np.float32)
    Wo = np.asarray(Wo, np.float32)
    bq = np.asarray(bq, np.float32)
    bk = np.asarray(bk, np.float32)
    bv = np.asarray(bv, np.float32)

    def wpack(w_rows):  # [128(m), D] -> [p, ko, m]
        return np.ascontiguousarray(
            w_rows.reshape(P, KO, P).transpose(2, 1, 0)
        ).astype(bf)

    in_maps = []
    for c in range(N_CORES):
        r = slice(c * P, (c + 1) * P)
        in_maps.append({
            "xT": xT,
            "wqT": wpack(Wq[r]),
            "wkT": wpack(Wk[r]),
            "wvT": wpack(Wv[r]),
            "woT": np.ascontiguousarray(Wo[:, r].T).astype(bf),
            "bqk": np.stack([bq[r], bk[r], bv[r]]),
            "bvrep": np.tile(bv[r][None, :], (P, 1)).astype(np.float32),
            "masks": masks,
        })
    return in_maps


def kernel(x, Wq, bq, Wk, bk, Wv, bv, Wo, bo):
    if "nc" not in _STATE:
        _STATE["nc"] = build()
    nc = _STATE["nc"]
    in_maps = _prep_inputs(x, Wq, bq, Wk, bk, Wv, bv, Wo, bo)
    res = run_bass_kernel_spmd(nc, in_maps, core_ids=list(range(N_CORES)))
    total = res.results[0]["outT"].astype(np.float32, copy=True)
    for c in range(1, N_CORES):
        total += res.results[c]["outT"]
    out = total.T + np.asarray(bo, np.float32)[None, :]
    return np.ascontiguousarray(out, dtype=np.float32).reshape(1, S, D)



# revision 52
# speedup vs baseline: 1.1522x; 1.1254x over previous
"""Multi-head causal self-attention (B=1, S=4096, D=1024, H=16) on 8 TRN2 cores.

Sharding: 2 heads per core (head/tensor parallel). Each core computes its
heads' Q/K/V projections, causal flash attention, and a partial output
projection against its 128 columns of Wo. The host sums the 8 partials and
adds the output bias.

Device layouts (per core, bf16 compute):
  - x is host-packed [p, nb, ko, nq] so each chunk DMA is partition-contiguous
  - Q^T, K^T [128, 4096]: per-core head dims on partitions (h0: 0-63, h1: 64-127)
  - V natural [4096, 130]: per seq-tile [128, 65*2] = [V_h0 | ones | V_h1 | ones]
    The ones column makes the PV matmul also produce the softmax denominator.
    V is projected with the xT chunk as the stationary operand (no transpose).
  - scores are computed transposed S^T[k, q] so the PV matmul needs no
    transposition; both heads' score matmuls run concurrently in the PE array
    (K=64 row tiling via base_partition). Softmax is exp-only (scores are
    bounded, no max-subtract); EXP on the Scalar engine is the per-tile pacer.
  - output is written transposed outT [1024, 4096] bf16 (partial; host sums
    the 8 partials in fp32 — costs ~3e-4 relative error, well inside budget).

Schedule: a rolling software pipeline over (block, key-tile): PV runs one
key tile behind scores/EXP and rolls across block boundaries; q/k/v
projections of block b+1 are paced with even deadlines through block b;
oproj m-tiles trickle at most one per key tile. A 60-matmul warmup holds the
PE busy through the input DMA wait so the HAM clock gate opens (2.4GHz)
before the first projection.
"""

import numpy as np
import ml_dtypes
from contextlib import ExitStack

import concourse.bass as bass
import concourse.tile as tile
from concourse import bacc, mybir
from concourse.bass_utils import run_bass_kernel_spmd

P = 128
S = 4096
D = 1024
DH = 64
N_CORES = 8
SCALE = 1.0 / 8.0  # 1/sqrt(64)
NQ = 512           # query block (matmul free dim)
KT = 128           # key tile (contraction partitions)
NQB = S // NQ      # 8 query blocks
NKT = S // KT      # 32 key tiles
KO = D // P        # 8 contraction subtiles over the model dim

BF16 = mybir.dt.bfloat16
F32 = mybir.dt.float32
EXP = mybir.ActivationFunctionType.Exp
ADD = mybir.AluOpType.add


def _emit(tc, xT, wqT, wkT, wvT, woT, bqk, bvrep, masks, outT, dbg=None):
    nc = tc.nc
    with ExitStack() as ctx:
        from collections import deque
        from concourse.masks import make_identity

        const = ctx.enter_context(tc.tile_pool(name="const", bufs=1))

        # weights/constants first: the first projections need wq + x chunk 0,
        # so nothing small may queue behind the 8.4MB xT transfer. All inputs
        # are host-prepacked to partition-contiguous layouts (128 descriptors
        # per transfer instead of 1024).
        wq_sb = const.tile([P, KO, P], BF16)
        nc.sync.dma_start(wq_sb, wqT)
        bqk_sb = const.tile([P, 3], F32)
        nc.sync.dma_start(bqk_sb, bqk.rearrange("b p -> p b"))
        xT_sb = const.tile([P, NQB, KO, NQ], BF16)
        nc.sync.dma_start(xT_sb[:, 0], xT[:, 0])  # q-proj of block 0 can start
        wk_sb = const.tile([P, KO, P], BF16)
        nc.sync.dma_start(wk_sb, wkT)
        wv_sb = const.tile([P, KO, P], BF16)
        nc.sync.dma_start(wv_sb, wvT)
        masks_sb = const.tile([P, KT], BF16)
        nc.sync.dma_start(masks_sb, masks)
        bv_sb = const.tile([P, P], F32)
        nc.sync.dma_start(bv_sb, bvrep)
        nc.sync.dma_start(xT_sb[:, 1], xT[:, 1])
        wo_sb = const.tile([P, D], BF16)
        nc.sync.dma_start(wo_sb, woT)
        for n in range(2, NQB):  # chunked so projections can start early
            nc.sync.dma_start(xT_sb[:, n], xT[:, n])

        qT_sb = const.tile([P, S], BF16)
        kT_sb = const.tile([P, S], BF16)
        v_sb = const.tile([P, S // P, 130], BF16)
        attnT_sb = const.tile([P, S], BF16)
        nc.vector.memset(v_sb, 1.0)  # presets the two ones-columns

        ident = const.tile([P, P], BF16)
        make_identity(nc, ident)

        # Warm the PE clock (HAM) with throwaway matmuls while the input DMAs
        # land. The HAM needs ~3.4us of *sustained* PE activity to unthrottle
        # (cold MMs run at 1.2GHz, so 60 x N=128 ~ 6.4us) — and the burst must
        # also bridge the DMA wait so the first projections start warm.
        with tc.tile_pool(name="warm_psum", bufs=1, space="PSUM") as wpool:
            wt = wpool.tile([P, P], F32)
            for _ in range(60):
                nc.tensor.matmul(wt, lhsT=ident, rhs=ident, start=True, stop=True)

        # PSUM budget (8 banks): spool 4 (two [128,2,512] score slabs),
        # vpool 2 (pv0/pv1 accumulators), ppool 1 (proj accum / V transpose),
        # opool 1 (output projection).
        spool = ctx.enter_context(tc.tile_pool(name="score_psum", bufs=2, space="PSUM"))
        vpool = ctx.enter_context(tc.tile_pool(name="pv_psum", bufs=1, space="PSUM"))
        ppool = ctx.enter_context(tc.tile_pool(name="proj_psum", bufs=1, space="PSUM"))
        opool = ctx.enter_context(tc.tile_pool(name="oproj_psum", bufs=1, space="PSUM"))
        work = ctx.enter_context(tc.tile_pool(name="work", bufs=7))
        nwork = ctx.enter_context(tc.tile_pool(name="nwork", bufs=3))
        dpool = ctx.enter_context(tc.tile_pool(name="dscratch", bufs=2, space="DRAM"))

        def proj_chunk(bcol, w_sb, dst, n):
            """Two pacing items of 4 accumulation matmuls each (shared psum)."""
            state = {}

            def emit_lo():
                ps = ppool.tile([P, NQ], F32, tag="ps", name=f"ps_{bcol}_{n}")
                state["ps"] = ps
                for kt in range(KO // 2):
                    nc.tensor.matmul(
                        ps,
                        lhsT=w_sb[:, kt, :],
                        rhs=xT_sb[:, n, kt, :],
                        start=(kt == 0),
                        stop=False,
                    )

            def emit_hi():
                ps = state["ps"]
                for kt in range(KO // 2, KO):
                    nc.tensor.matmul(
                        ps,
                        lhsT=w_sb[:, kt, :],
                        rhs=xT_sb[:, n, kt, :],
                        start=False,
                        stop=(kt == KO - 1),
                    )
                nc.vector.tensor_tensor(
                    dst[:, n * NQ:(n + 1) * NQ],
                    ps,
                    bqk_sb[:, bcol:bcol + 1].to_broadcast([P, NQ]),
                    op=ADD,
                )

            return [emit_lo, emit_hi]

        def v_proj(t):
            """V in natural [seq, dh] layout: stationary operand is the xT
            chunk, so no PE transpose is needed afterwards. One seq-tile of
            128 rows lands as [128, 2, 64] inside v_sb (ones columns preset)."""
            def emit():
                ps = ppool.tile([P, P], F32, tag="ps", name=f"vp_{t}")
                for kt in range(KO):
                    nc.tensor.matmul(
                        ps,
                        lhsT=xT_sb[:, t // 4, kt, (t % 4) * P:(t % 4 + 1) * P],
                        rhs=wv_sb[:, kt, :],
                        start=(kt == 0),
                        stop=(kt == KO - 1),
                    )
                nc.vector.tensor_tensor(
                    v_sb[:, t, :].rearrange("p (h x) -> p h x", x=65)[:, :, 0:DH],
                    ps.rearrange("p (h x) -> p h x", x=DH),
                    bv_sb.rearrange("p (h x) -> p h x", x=DH),
                    op=ADD,
                )
            return emit

        def proj_ops(nb):
            ops = []
            ops += proj_chunk(0, wq_sb, qT_sb, nb)
            ops += proj_chunk(1, wk_sb, kT_sb, nb)
            ops += [v_proj(t) for t in range(4 * nb, 4 * nb + 4)]
            return ops

        def oproj_mtile(b, m, tail=False):
            def emit():
                qsl = slice(b * NQ, (b + 1) * NQ)
                # in the drain after the last block, alternate PSUM banks and
                # cast engines so consecutive m-tiles don't serialize on the
                # single oproj bank / the DVE
                use_p = tail and (m % 2 == 1)
                pool, tag = (ppool, "ps") if use_p else (opool, "po")
                po = pool.tile([P, NQ], F32, tag=tag, name=f"po_{b}_{m}")
                nc.tensor.matmul(
                    po,
                    lhsT=wo_sb[:, m * P:(m + 1) * P],
                    rhs=attnT_sb[:, qsl],
                    start=True,
                    stop=True,
                )
                ot = work.tile([P, NQ], BF16, tag="ot", name=f"ot_{b}_{m}")
                if use_p:
                    nc.scalar.copy(ot, po)
                else:
                    nc.vector.tensor_copy(ot, po)
                nc.sync.dma_start(
                    outT.rearrange("(mo p) n -> p mo n", p=P)[:, m, qsl], ot
                )
            return emit

        # block 0's q/k projections up front; its v tiles are only needed by
        # the PV matmuls and would delay the first scores/EXP, so they go
        # into block 0's paced queue instead.
        for op in proj_chunk(0, wq_sb, qT_sb, 0) + proj_chunk(1, wk_sb, kT_sb, 0):
            op()

        ones1 = const.tile([1, DH], F32)
        nc.vector.memset(ones1, 1.0)

        def make_norm(b, pvs, tail=False):
            """Normalize block b's PV accumulators into attnT. Heads are
            interleaved so the DVE works on head 1 while head 0's reciprocal
            broadcast makes its DRAM round-trip. In the tail (last block) the
            broadcast is a K=1 PE matmul instead — the PE is idle there and
            it avoids ~2us of DMA latency on the critical path."""
            def norm():
                qsl = slice(b * NQ, (b + 1) * NQ)
                pvS_t, rb_t = [], []
                for h in (0, 1):
                    if tail:
                        # denominator row first: the reciprocal + broadcast
                        # then overlap the bulk PSUM->SBUF copy
                        rcp0 = nwork.tile([1, NQ], F32, tag=f"rcp0{h}")
                        nc.vector.tensor_copy(rcp0, pvs[h][DH:DH + 1, :])
                        nc.vector.reciprocal_approx_fast(rcp0, rcp0)
                        pvS = nwork.tile([DH + 1, NQ], F32, tag=f"pvS{h}")
                        nc.vector.tensor_copy(pvS, pvs[h])
                    else:
                        # PSUM-freeing copy first: the next block's first PV
                        # matmul reuses this bank and must not wait
                        pvS = nwork.tile([DH + 1, NQ], F32, tag=f"pvS{h}")
                        nc.vector.tensor_copy(pvS, pvs[h])
                        rcp0 = nwork.tile([1, NQ], F32, tag=f"rcp0{h}")
                        nc.vector.tensor_copy(rcp0, pvS[DH:DH + 1, :])
                        nc.vector.reciprocal_approx_fast(rcp0, rcp0)
                    if tail:
                        rb = vpool.tile([DH + 1, NQ], F32, tag=f"pv{h}",
                                        name=f"rb_{b}_{h}")
                        nc.tensor.matmul(rb[0:DH, :], lhsT=ones1, rhs=rcp0,
                                         start=True, stop=True)
                    else:
                        scr = dpool.tile([NQ], F32, tag=f"scr{h}")
                        nc.sync.dma_start(scr, rcp0)
                        rb = nwork.tile([DH, NQ], F32, tag=f"rb{h}")
                        nc.sync.dma_start(rb, scr[None, :].to_broadcast([DH, NQ]))
                    pvS_t.append(pvS)
                    rb_t.append(rb)
                for h in (0, 1):
                    nc.vector.tensor_mul(attnT_sb[h * DH:(h + 1) * DH, qsl],
                                         pvS_t[h][0:DH, :], rb_t[h][0:DH, :])
                    if dbg is not None and b == 1:
                        nc.sync.dma_start(dbg[f"pv{h}"], pvS_t[h])
            return norm

        soft_q = deque()  # oproj items: no hard deadline, trickled 1 per kt
        prev = None       # (emit_pv, state): PV runs one k-tile behind, and
                          # rolls across block boundaries to keep the PE fed
        norm_fn = None    # previous block's normalization, emitted after the
                          # next block's first scores/exp so ACT never waits
        for b in range(NQB):
            nk = 4 * (b + 1)  # causal: only key tiles up to the diagonal
            # q/k/v projections of block b+1 must finish during block b; give
            # each item an even deadline so there is no burst at the boundary.
            hard_q = deque(proj_ops(b + 1)) if b + 1 < NQB else deque()
            if b == 0:
                hard_q.extendleft(reversed([v_proj(t) for t in range(4)]))
            nhard = len(hard_q)
            if b > 0:
                soft_q.extend(oproj_mtile(b - 1, m) for m in range(D // P))
            pvs = [
                vpool.tile([DH + 1, NQ], F32, tag=f"pv{h}", name=f"pv{h}_{b}")
                for h in (0, 1)
            ]

            def emit_pv(st, pvs=pvs, nk=nk):
                pT, kt, q0, nq = st
                for h in (0, 1):
                    nc.tensor.matmul(
                        pvs[h][:, q0:],
                        lhsT=v_sb[:, kt, h * 65:(h + 1) * 65],
                        rhs=pT[:, h, :nq],
                        start=(kt == 0),
                        stop=(kt == nk - 1),
                    )

            for kt in range(nk):
                j = kt - 4 * b  # >= 0 on causal-diagonal key tiles
                # on diagonal tiles only queries >= 128j can attend this tile
                q0 = max(0, j) * KT
                nq = NQ - q0
                qs0 = b * NQ + q0
                # mid-block: PV of the previous tile goes first (it is ready —
                # scores may still wait on its slab). At a block boundary the
                # rolled-over PV waits on the last diagonal mask, so scores
                # go first to keep ACT fed across the boundary.
                if prev is not None and kt > 0:
                    prev[0](prev[1])
                    prev = None
                slab = spool.tile([P, 2, NQ], F32, tag="slab")
                for h in (0, 1):
                    nc.tensor.matmul(
                        slab[:, h, :nq],
                        lhsT=kT_sb[h * DH:(h + 1) * DH, kt * KT:(kt + 1) * KT],
                        rhs=qT_sb[h * DH:(h + 1) * DH, qs0:qs0 + nq],
                        start=True,
                        stop=True,
                    )
                if prev is not None:
                    prev[0](prev[1])
                    prev = None
                pT = work.tile([P, 2, NQ], BF16, tag="pT")
                nc.scalar.activation(pT[:, :, :nq], slab[:, :, :nq], EXP, scale=SCALE)
                if j >= 0:
                    # causal mask: within a diagonal tile only the first 128
                    # queries (qq < 128 relative to q0) form a triangle with
                    # the 128 keys; later queries see the whole tile.
                    nc.vector.tensor_mul(
                        pT[:, :, 0:KT],
                        pT[:, :, 0:KT],
                        masks_sb.rearrange("p (o q) -> p o q", o=1).to_broadcast(
                            [P, 2, KT]
                        ),
                    )
                if norm_fn is not None:
                    norm_fn()
                    norm_fn = None
                prev = (emit_pv, (pT, kt, q0, nq))
                # hard items: evenly spread, finishing ~2 tiles before the
                # boundary so the last bias-add isn't racing the next block's
                # first scores on the DVE
                done_hard = nhard - len(hard_q)
                due = -(-nhard * (kt + 1) // max(1, nk - 2))
                while hard_q and done_hard < due:
                    hard_q.popleft()()
                    done_hard += 1
                # at most one oproj per key tile (single-buffered PSUM bank),
                # and only once the previous block's attnT normalization has
                # had time to finish — else its first matmul blocks the PE FIFO
                if soft_q and kt >= 4:
                    soft_q.popleft()()
            while hard_q:
                hard_q.popleft()()
            norm_fn = make_norm(b, pvs, tail=(b == NQB - 1))
        prev[0](prev[1])
        norm_fn()
        while soft_q:
            soft_q.popleft()()
        for m in range(D // P):
            oproj_mtile(NQB - 1, m, tail=True)()
        if dbg is not None:
            nc.sync.dma_start(dbg["qT"], qT_sb)
            nc.sync.dma_start(dbg["kT"], kT_sb)
            nc.sync.dma_start(dbg["v"], v_sb)
            nc.sync.dma_start(dbg["attnT"], attnT_sb)


def build(debug_out=False):
    nc = bacc.Bacc(
        "TRN2",
        target_bir_lowering=False,
        debug=False,
        enable_asserts=False,
    )
    xT = nc.dram_tensor("xT", [P, NQB, KO, NQ], BF16, kind="ExternalInput").ap()
    wqT = nc.dram_tensor("wqT", [P, KO, P], BF16, kind="ExternalInput").ap()
    wkT = nc.dram_tensor("wkT", [P, KO, P], BF16, kind="ExternalInput").ap()
    wvT = nc.dram_tensor("wvT", [P, KO, P], BF16, kind="ExternalInput").ap()
    woT = nc.dram_tensor("woT", [P, D], BF16, kind="ExternalInput").ap()
    bqk = nc.dram_tensor("bqk", [3, P], F32, kind="ExternalInput").ap()
    bvrep = nc.dram_tensor("bvrep", [P, P], F32, kind="ExternalInput").ap()
    masks = nc.dram_tensor("masks", [P, KT], BF16, kind="ExternalInput").ap()
    outT = nc.dram_tensor("outT", [D, S], BF16, kind="ExternalOutput").ap()
    dbg = None
    if debug_out:
        dbg = {
            "qT": nc.dram_tensor("dbg_qT", [P, S], BF16, kind="ExternalOutput").ap(),
            "kT": nc.dram_tensor("dbg_kT", [P, S], BF16, kind="ExternalOutput").ap(),
            "v": nc.dram_tensor("dbg_v", [P, S // P, 130], BF16, kind="ExternalOutput").ap(),
            "attnT": nc.dram_tensor("dbg_attnT", [P, S], BF16, kind="ExternalOutput").ap(),
            "pv0": nc.dram_tensor("dbg_pv0", [DH + 1, NQ], F32, kind="ExternalOutput").ap(),
            "pv1": nc.dram_tensor("dbg_pv1", [DH + 1, NQ], F32, kind="ExternalOutput").ap(),
        }

    with tile.TileContext(nc) as tc:
        _emit(tc, xT, wqT, wkT, wvT, woT, bqk, bvrep, masks, outT, dbg=dbg)
    nc.compile()
    return nc


def _make_masks():
    k = np.arange(P)[:, None]
    q = np.arange(KT)[None, :]
    return (k <= q).astype(ml_dtypes.bfloat16)


_STATE = {}


def _prep_inputs(x, Wq, bq, Wk, bk, Wv, bv, Wo, bo):
    bf = ml_dtypes.bfloat16
    x2 = np.asarray(x, np.float32).reshape(NQB, NQ, KO, P)
    # xT[p, nb, ko, nq] = x[nb*NQ+nq, ko*P+p]: partition-contiguous chunks
    xT = np.ascontiguousarray(x2.transpose(3, 0, 2, 1)).astype(bf)
    masks = _make_masks()
    Wq = np.asarray(Wq, np.float32)
    Wk = np.asarray(Wk, np.float32)
    Wv = np.asarray(Wv, np.float32)
    Wo = np.asarray(Wo, np.float32)
    bq = np.asarray(bq, np.float32)
    bk = np.asarray(bk, np.float32)
    bv = np.asarray(bv, np.float32)

    def wpack(w_rows):  # [128(m), D] -> [p, ko, m]
        return np.ascontiguousarray(
            w_rows.reshape(P, KO, P).transpose(2, 1, 0)
        ).astype(bf)

    in_maps = []
    for c in range(N_CORES):
        r = slice(c * P, (c + 1) * P)
        in_maps.append({
            "xT": xT,
            "wqT": wpack(Wq[r]),
            "wkT": wpack(Wk[r]),
            "wvT": wpack(Wv[r]),
            "woT": np.ascontiguousarray(Wo[:, r].T).astype(bf),
            "bqk": np.stack([bq[r], bk[r], bv[r]]),
            "bvrep": np.tile(bv[r][None, :], (P, 1)).astype(np.float32),
            "masks": masks,
        })
    return in_maps


def kernel(x, Wq, bq, Wk, bk, Wv, bv, Wo, bo):
    if "nc" not in _STATE:
        _STATE["nc"] = build()
    nc = _STATE["nc"]
    in_maps = _prep_inputs(x, Wq, bq, Wk, bk, Wv, bv, Wo, bo)
    res = run_bass_kernel_spmd(nc, in_maps, core_ids=list(range(N_CORES)))
    total = res.results[0]["outT"].astype(np.float32, copy=True)
    for c in range(1, N_CORES):
        total += res.results[c]["outT"]
    out = total.T + np.asarray(bo, np.float32)[None, :]
    return np.ascontiguousarray(out, dtype=np.float32).reshape(1, S, D)



# revision 53
# speedup vs baseline: 1.1716x; 1.0169x over previous
"""Multi-head causal self-attention (B=1, S=4096, D=1024, H=16) on 8 TRN2 cores.

Sharding: 2 heads per core (head/tensor parallel). Each core computes its
heads' Q/K/V projections, causal flash attention, and a partial output
projection against its 128 columns of Wo. The host sums the 8 partials and
adds the output bias.

Device layouts (per core, bf16 compute):
  - x is host-packed [p, nb, ko, nq] so each chunk DMA is partition-contiguous
  - Q^T, K^T [128, 4096]: per-core head dims on partitions (h0: 0-63, h1: 64-127)
  - V natural [4096, 130]: per seq-tile [128, 65*2] = [V_h0 | ones | V_h1 | ones]
    The ones column makes the PV matmul also produce the softmax denominator.
    V is projected with the xT chunk as the stationary operand (no transpose).
  - scores are computed transposed S^T[k, q] so the PV matmul needs no
    transposition; both heads' score matmuls run concurrently in the PE array
    (K=64 row tiling via base_partition). Softmax is exp-only (scores are
    bounded, no max-subtract); EXP on the Scalar engine is the per-tile pacer.
  - output is written transposed outT [1024, 4096] bf16 (partial; host sums
    the 8 partials in fp32 — costs ~3e-4 relative error, well inside budget).

Schedule: a rolling software pipeline over (block, key-tile): PV runs one
key tile behind scores/EXP and rolls across block boundaries; q/k/v
projections of block b+1 are paced with even deadlines through block b;
oproj m-tiles trickle at most one per key tile. A 60-matmul warmup holds the
PE busy through the input DMA wait so the HAM clock gate opens (2.4GHz)
before the first projection.
"""

import numpy as np
import ml_dtypes
from contextlib import ExitStack

import concourse.bass as bass
import concourse.tile as tile
from concourse import bacc, mybir
from concourse.bass_utils import run_bass_kernel_spmd

P = 128
S = 4096
D = 1024
DH = 64
N_CORES = 8
SCALE = 1.0 / 8.0  # 1/sqrt(64)
NQ = 512           # query block (matmul free dim)
KT = 128           # key tile (contraction partitions)
NQB = S // NQ      # 8 query blocks
NKT = S // KT      # 32 key tiles
KO = D // P        # 8 contraction subtiles over the model dim

BF16 = mybir.dt.bfloat16
F32 = mybir.dt.float32
EXP = mybir.ActivationFunctionType.Exp
ADD = mybir.AluOpType.add


def _emit(tc, xT, wqT, wkT, wvT, woT, bqk, bvrep, masks, outT, dbg=None):
    nc = tc.nc
    with ExitStack() as ctx:
        from collections import deque
        from concourse.masks import make_identity

        const = ctx.enter_context(tc.tile_pool(name="const", bufs=1))

        # weights/constants first: the first projections need wq + x chunk 0,
        # so nothing small may queue behind the 8.4MB xT transfer. All inputs
        # are host-prepacked to partition-contiguous layouts (128 descriptors
        # per transfer instead of 1024).
        wq_sb = const.tile([P, KO, P], BF16)
        nc.sync.dma_start(wq_sb, wqT)
        bqk_sb = const.tile([P, 3], F32)
        nc.sync.dma_start(bqk_sb, bqk.rearrange("b p -> p b"))
        xT_sb = const.tile([P, NQB, KO, NQ], BF16)
        nc.sync.dma_start(xT_sb[:, 0], xT[:, 0])  # q-proj of block 0 can start
        wk_sb = const.tile([P, KO, P], BF16)
        nc.sync.dma_start(wk_sb, wkT)
        wv_sb = const.tile([P, KO, P], BF16)
        nc.sync.dma_start(wv_sb, wvT)
        masks_sb = const.tile([P, KT], BF16)
        nc.sync.dma_start(masks_sb, masks)
        bv_sb = const.tile([P, P], F32)
        nc.sync.dma_start(bv_sb, bvrep)
        nc.sync.dma_start(xT_sb[:, 1], xT[:, 1])
        wo_sb = const.tile([P, D], BF16)
        nc.sync.dma_start(wo_sb, woT)
        for n in range(2, NQB):  # chunked so projections can start early
            nc.sync.dma_start(xT_sb[:, n], xT[:, n])

        qT_sb = const.tile([P, S], BF16)
        kT_sb = const.tile([P, S], BF16)
        v_sb = const.tile([P, S // P, 130], BF16)
        attnT_sb = const.tile([P, S], BF16)
        nc.vector.memset(v_sb, 1.0)  # presets the two ones-columns

        ident = const.tile([P, P], BF16)
        make_identity(nc, ident)

        # Warm the PE clock (HAM) with throwaway matmuls while the input DMAs
        # land. The HAM needs ~3.4us of *sustained* PE activity to unthrottle
        # (cold MMs run at 1.2GHz, so 60 x N=128 ~ 6.4us) — and the burst must
        # also bridge the DMA wait so the first projections start warm.
        with tc.tile_pool(name="warm_psum", bufs=1, space="PSUM") as wpool:
            wt = wpool.tile([P, P], F32)
            for _ in range(60):
                nc.tensor.matmul(wt, lhsT=ident, rhs=ident, start=True, stop=True)

        # PSUM budget (8 banks): spool 4 (two [128,2,512] score slabs),
        # vpool 2 (pv0/pv1 accumulators), ppool 1 (proj accum / V transpose),
        # opool 1 (output projection).
        spool = ctx.enter_context(tc.tile_pool(name="score_psum", bufs=2, space="PSUM"))
        vpool = ctx.enter_context(tc.tile_pool(name="pv_psum", bufs=1, space="PSUM"))
        ppool = ctx.enter_context(tc.tile_pool(name="proj_psum", bufs=1, space="PSUM"))
        opool = ctx.enter_context(tc.tile_pool(name="oproj_psum", bufs=1, space="PSUM"))
        work = ctx.enter_context(tc.tile_pool(name="work", bufs=7))
        nwork = ctx.enter_context(tc.tile_pool(name="nwork", bufs=4))
        dpool = ctx.enter_context(tc.tile_pool(name="dscratch", bufs=4, space="DRAM"))

        def proj_chunk(bcol, w_sb, dst, n):
            """Two pacing items of 4 accumulation matmuls each (shared psum)."""
            state = {}

            def emit_lo():
                ps = ppool.tile([P, NQ], F32, tag="ps", name=f"ps_{bcol}_{n}")
                state["ps"] = ps
                for kt in range(KO // 2):
                    nc.tensor.matmul(
                        ps,
                        lhsT=w_sb[:, kt, :],
                        rhs=xT_sb[:, n, kt, :],
                        start=(kt == 0),
                        stop=False,
                    )

            def emit_hi():
                ps = state["ps"]
                for kt in range(KO // 2, KO):
                    nc.tensor.matmul(
                        ps,
                        lhsT=w_sb[:, kt, :],
                        rhs=xT_sb[:, n, kt, :],
                        start=False,
                        stop=(kt == KO - 1),
                    )
                nc.vector.tensor_tensor(
                    dst[:, n * NQ:(n + 1) * NQ],
                    ps,
                    bqk_sb[:, bcol:bcol + 1].to_broadcast([P, NQ]),
                    op=ADD,
                )

            return [emit_lo, emit_hi]

        def v_proj(t):
            """V in natural [seq, dh] layout: stationary operand is the xT
            chunk, so no PE transpose is needed afterwards. One seq-tile of
            128 rows lands as [128, 2, 64] inside v_sb (ones columns preset)."""
            def emit():
                ps = ppool.tile([P, P], F32, tag="ps", name=f"vp_{t}")
                for kt in range(KO):
                    nc.tensor.matmul(
                        ps,
                        lhsT=xT_sb[:, t // 4, kt, (t % 4) * P:(t % 4 + 1) * P],
                        rhs=wv_sb[:, kt, :],
                        start=(kt == 0),
                        stop=(kt == KO - 1),
                    )
                nc.vector.tensor_tensor(
                    v_sb[:, t, :].rearrange("p (h x) -> p h x", x=65)[:, :, 0:DH],
                    ps.rearrange("p (h x) -> p h x", x=DH),
                    bv_sb.rearrange("p (h x) -> p h x", x=DH),
                    op=ADD,
                )
            return emit

        def proj_ops(nb):
            ops = []
            ops += proj_chunk(0, wq_sb, qT_sb, nb)
            ops += proj_chunk(1, wk_sb, kT_sb, nb)
            ops += [v_proj(t) for t in range(4 * nb, 4 * nb + 4)]
            return ops

        def oproj_mtile(b, m, tail=False):
            def emit():
                qsl = slice(b * NQ, (b + 1) * NQ)
                # in the drain after the last block, alternate PSUM banks and
                # cast engines so consecutive m-tiles don't serialize on the
                # single oproj bank / the DVE
                use_p = tail and (m % 2 == 1)
                pool, tag = (ppool, "ps") if use_p else (opool, "po")
                po = pool.tile([P, NQ], F32, tag=tag, name=f"po_{b}_{m}")
                nc.tensor.matmul(
                    po,
                    lhsT=wo_sb[:, m * P:(m + 1) * P],
                    rhs=attnT_sb[:, qsl],
                    start=True,
                    stop=True,
                )
                ot = work.tile([P, NQ], BF16, tag="ot", name=f"ot_{b}_{m}")
                if use_p:
                    nc.scalar.copy(ot, po)
                else:
                    nc.vector.tensor_copy(ot, po)
                nc.sync.dma_start(
                    outT.rearrange("(mo p) n -> p mo n", p=P)[:, m, qsl], ot
                )
            return emit

        # block 0's q/k projections up front; its v tiles are only needed by
        # the PV matmuls and would delay the first scores/EXP, so they go
        # into block 0's paced queue instead.
        for op in proj_chunk(0, wq_sb, qT_sb, 0) + proj_chunk(1, wk_sb, kT_sb, 0):
            op()

        ones1 = const.tile([1, DH], F32)
        nc.vector.memset(ones1, 1.0)

        def make_norm(b, pvs, tail=False):
            """Normalize block b's PV accumulators into attnT. Heads are
            interleaved so the DVE works on head 1 while head 0's reciprocal
            broadcast makes its DRAM round-trip. In the tail (last block) the
            broadcast is a K=1 PE matmul instead — the PE is idle there and
            it avoids ~2us of DMA latency on the critical path."""
            def norm():
                qsl = slice(b * NQ, (b + 1) * NQ)
                pvS_t, rb_t = [], []
                for h in (0, 1):
                    if tail:
                        # denominator row first: the reciprocal + broadcast
                        # then overlap the bulk PSUM->SBUF copy
                        rcp0 = nwork.tile([1, NQ], F32, tag=f"rcp0{h}")
                        nc.vector.tensor_copy(rcp0, pvs[h][DH:DH + 1, :])
                        nc.vector.reciprocal_approx_fast(rcp0, rcp0)
                        pvS = nwork.tile([DH + 1, NQ], F32, tag=f"pvS{h}")
                        nc.vector.tensor_copy(pvS, pvs[h])
                    else:
                        # PSUM-freeing copy first: the next block's first PV
                        # matmul reuses this bank and must not wait
                        pvS = nwork.tile([DH + 1, NQ], F32, tag=f"pvS{h}")
                        nc.vector.tensor_copy(pvS, pvs[h])
                        rcp0 = nwork.tile([1, NQ], F32, tag=f"rcp0{h}")
                        nc.vector.tensor_copy(rcp0, pvS[DH:DH + 1, :])
                        nc.vector.reciprocal_approx_fast(rcp0, rcp0)
                    if tail:
                        rb = vpool.tile([DH + 1, NQ], F32, tag=f"pv{h}",
                                        name=f"rb_{b}_{h}")
                        nc.tensor.matmul(rb[0:DH, :], lhsT=ones1, rhs=rcp0,
                                         start=True, stop=True)
                    else:
                        scr = dpool.tile([NQ], F32, tag=f"scr{h}")
                        nc.sync.dma_start(scr, rcp0)
                        rb = nwork.tile([DH, NQ], F32, tag=f"rb{h}")
                        nc.sync.dma_start(rb, scr[None, :].to_broadcast([DH, NQ]))
                    pvS_t.append(pvS)
                    rb_t.append(rb)
                for h in (0, 1):
                    nc.vector.tensor_mul(attnT_sb[h * DH:(h + 1) * DH, qsl],
                                         pvS_t[h][0:DH, :], rb_t[h][0:DH, :])
                    if dbg is not None and b == 1:
                        nc.sync.dma_start(dbg[f"pv{h}"], pvS_t[h])
            return norm

        soft_q = deque()  # oproj items: no hard deadline, trickled 1 per kt
        prev = None       # (emit_pv, state): PV runs one k-tile behind, and
                          # rolls across block boundaries to keep the PE fed
        norm_fn = None    # previous block's normalization, emitted after the
                          # next block's first scores/exp so ACT never waits
        for b in range(NQB):
            nk = 4 * (b + 1)  # causal: only key tiles up to the diagonal
            # q/k/v projections of block b+1 must finish during block b; give
            # each item an even deadline so there is no burst at the boundary.
            hard_q = deque(proj_ops(b + 1)) if b + 1 < NQB else deque()
            if b == 0:
                hard_q.extendleft(reversed([v_proj(t) for t in range(4)]))
            nhard = len(hard_q)
            if b > 0:
                soft_q.extend(oproj_mtile(b - 1, m) for m in range(D // P))
            pvs = [
                vpool.tile([DH + 1, NQ], F32, tag=f"pv{h}", name=f"pv{h}_{b}")
                for h in (0, 1)
            ]

            def emit_pv(st, pvs=pvs, nk=nk):
                pT, kt, q0, nq = st
                for h in (0, 1):
                    nc.tensor.matmul(
                        pvs[h][:, q0:],
                        lhsT=v_sb[:, kt, h * 65:(h + 1) * 65],
                        rhs=pT[:, h, :nq],
                        start=(kt == 0),
                        stop=(kt == nk - 1),
                    )

            for kt in range(nk):
                j = kt - 4 * b  # >= 0 on causal-diagonal key tiles
                # on diagonal tiles only queries >= 128j can attend this tile
                q0 = max(0, j) * KT
                nq = NQ - q0
                qs0 = b * NQ + q0
                # mid-block: PV of the previous tile goes first (it is ready —
                # scores may still wait on its slab). At a block boundary the
                # rolled-over PV waits on the last diagonal mask, so scores
                # go first to keep ACT fed across the boundary.
                if prev is not None and kt > 0:
                    prev[0](prev[1])
                    prev = None
                slab = spool.tile([P, 2, NQ], F32, tag="slab")
                for h in (0, 1):
                    nc.tensor.matmul(
                        slab[:, h, :nq],
                        lhsT=kT_sb[h * DH:(h + 1) * DH, kt * KT:(kt + 1) * KT],
                        rhs=qT_sb[h * DH:(h + 1) * DH, qs0:qs0 + nq],
                        start=True,
                        stop=True,
                    )
                if prev is not None:
                    prev[0](prev[1])
                    prev = None
                pT = work.tile([P, 2, NQ], BF16, tag="pT")
                nc.scalar.activation(pT[:, :, :nq], slab[:, :, :nq], EXP, scale=SCALE)
                if j >= 0:
                    # causal mask: within a diagonal tile only the first 128
                    # queries (qq < 128 relative to q0) form a triangle with
                    # the 128 keys; later queries see the whole tile.
                    nc.vector.tensor_mul(
                        pT[:, :, 0:KT],
                        pT[:, :, 0:KT],
                        masks_sb.rearrange("p (o q) -> p o q", o=1).to_broadcast(
                            [P, 2, KT]
                        ),
                    )
                if norm_fn is not None:
                    norm_fn()
                    norm_fn = None
                prev = (emit_pv, (pT, kt, q0, nq))
                # hard items: evenly spread, finishing ~2 tiles before the
                # boundary so the last bias-add isn't racing the next block's
                # first scores on the DVE
                done_hard = nhard - len(hard_q)
                due = -(-nhard * (kt + 1) // max(1, nk - 2))
                while hard_q and done_hard < due:
                    hard_q.popleft()()
                    done_hard += 1
                # at most one oproj per key tile (single-buffered PSUM bank),
                # and only once the previous block's attnT normalization has
                # had time to finish — else its first matmul blocks the PE FIFO
                if soft_q and kt >= 4:
                    soft_q.popleft()()
            while hard_q:
                hard_q.popleft()()
            norm_fn = make_norm(b, pvs, tail=(b == NQB - 1))
        prev[0](prev[1])
        norm_fn()
        while soft_q:
            soft_q.popleft()()
        for m in range(D // P):
            oproj_mtile(NQB - 1, m, tail=True)()
        if dbg is not None:
            nc.sync.dma_start(dbg["qT"], qT_sb)
            nc.sync.dma_start(dbg["kT"], kT_sb)
            nc.sync.dma_start(dbg["v"], v_sb)
            nc.sync.dma_start(dbg["attnT"], attnT_sb)


def build(debug_out=False):
    nc = bacc.Bacc(
        "TRN2",
        target_bir_lowering=False,
        debug=False,
        enable_asserts=False,
    )
    xT = nc.dram_tensor("xT", [P, NQB, KO, NQ], BF16, kind="ExternalInput").ap()
    wqT = nc.dram_tensor("wqT", [P, KO, P], BF16, kind="ExternalInput").ap()
    wkT = nc.dram_tensor("wkT", [P, KO, P], BF16, kind="ExternalInput").ap()
    wvT = nc.dram_tensor("wvT", [P, KO, P], BF16, kind="ExternalInput").ap()
    woT = nc.dram_tensor("woT", [P, D], BF16, kind="ExternalInput").ap()
    bqk = nc.dram_tensor("bqk", [3, P], F32, kind="ExternalInput").ap()
    bvrep = nc.dram_tensor("bvrep", [P, P], F32, kind="ExternalInput").ap()
    masks = nc.dram_tensor("masks", [P, KT], BF16, kind="ExternalInput").ap()
    outT = nc.dram_tensor("outT", [D, S], BF16, kind="ExternalOutput").ap()
    dbg = None
    if debug_out:
        dbg = {
            "qT": nc.dram_tensor("dbg_qT", [P, S], BF16, kind="ExternalOutput").ap(),
            "kT": nc.dram_tensor("dbg_kT", [P, S], BF16, kind="ExternalOutput").ap(),
            "v": nc.dram_tensor("dbg_v", [P, S // P, 130], BF16, kind="ExternalOutput").ap(),
            "attnT": nc.dram_tensor("dbg_attnT", [P, S], BF16, kind="ExternalOutput").ap(),
            "pv0": nc.dram_tensor("dbg_pv0", [DH + 1, NQ], F32, kind="ExternalOutput").ap(),
            "pv1": nc.dram_tensor("dbg_pv1", [DH + 1, NQ], F32, kind="ExternalOutput").ap(),
        }

    with tile.TileContext(nc) as tc:
        _emit(tc, xT, wqT, wkT, wvT, woT, bqk, bvrep, masks, outT, dbg=dbg)
    nc.compile()
    return nc


def _make_masks():
    k = np.arange(P)[:, None]
    q = np.arange(KT)[None, :]
    return (k <= q).astype(ml_dtypes.bfloat16)


_STATE = {}


def _prep_inputs(x, Wq, bq, Wk, bk, Wv, bv, Wo, bo):
    bf = ml_dtypes.bfloat16
    x2 = np.asarray(x, np.float32).reshape(NQB, NQ, KO, P)
    # xT[p, nb, ko, nq] = x[nb*NQ+nq, ko*P+p]: partition-contiguous chunks
    xT = np.ascontiguousarray(x2.transpose(3, 0, 2, 1)).astype(bf)
    masks = _make_masks()
    Wq = np.asarray(Wq, np.float32)
    Wk = np.asarray(Wk, np.float32)
    Wv = np.asarray(Wv, np.float32)
    Wo = np.asarray(Wo, np.float32)
    bq = np.asarray(bq, np.float32)
    bk = np.asarray(bk, np.float32)
    bv = np.asarray(bv, np.float32)

    def wpack(w_rows):  # [128(m), D] -> [p, ko, m]
        return np.ascontiguousarray(
            w_rows.reshape(P, KO, P).transpose(2, 1, 0)
        ).astype(bf)

    in_maps = []
    for c in range(N_CORES):
        r = slice(c * P, (c + 1) * P)
        in_maps.append({
            "xT": xT,
            "wqT": wpack(Wq[r]),
            "wkT": wpack(Wk[r]),
            "wvT": wpack(Wv[r]),
            "woT": np.ascontiguousarray(Wo[:, r].T).astype(bf),
            "bqk": np.stack([bq[r], bk[r], bv[r]]),
            "bvrep": np.tile(bv[r][None, :], (P, 1)).astype(np.float32),
            "masks": masks,
        })
    return in_maps


def kernel(x, Wq, bq, Wk, bk, Wv, bv, Wo, bo):
    if "nc" not in _STATE:
        _STATE["nc"] = build()
    nc = _STATE["nc"]
    in_maps = _prep_inputs(x, Wq, bq, Wk, bk, Wv, bv, Wo, bo)
    res = run_bass_kernel_spmd(nc, in_maps, core_ids=list(range(N_CORES)))
    total = res.results[0]["outT"].astype(np.float32, copy=True)
    for c in range(1, N_CORES):
        total += res.results[c]["outT"]
    out = total.T + np.asarray(bo, np.float32)[None, :]
    return np.ascontiguousarray(out, dtype=np.float32).reshape(1, S, D)



# revision 54
# speedup vs baseline: 1.2139x; 1.0360x over previous
"""Multi-head causal self-attention (B=1, S=4096, D=1024, H=16) on 8 TRN2 cores.

Sharding: 2 heads per core (head/tensor parallel). Each core computes its
heads' Q/K/V projections, causal flash attention, and a partial output
projection against its 128 columns of Wo. The host sums the 8 partials and
adds the output bias.

Device layouts (per core, bf16 compute):
  - x is host-packed [p, nb, ko, nq] so each chunk DMA is partition-contiguous
  - Q^T, K^T [128, 4096]: per-core head dims on partitions (h0: 0-63, h1: 64-127)
  - V natural [4096, 130]: per seq-tile [128, 65*2] = [V_h0 | ones | V_h1 | ones]
    The ones column makes the PV matmul also produce the softmax denominator.
    V is projected with the xT chunk as the stationary operand (no transpose).
  - scores are computed transposed S^T[k, q] so the PV matmul needs no
    transposition; both heads' score matmuls run concurrently in the PE array
    (K=64 row tiling via base_partition). Softmax is exp-only (scores are
    bounded, no max-subtract); EXP on the Scalar engine is the per-tile pacer.
  - output is written transposed outT [1024, 4096] bf16 (partial; host sums
    the 8 partials in fp32 — costs ~3e-4 relative error, well inside budget).

Schedule: a rolling software pipeline over (block, key-tile): PV runs one
key tile behind scores/EXP and rolls across block boundaries; q/k/v
projections of block b+1 are paced with even deadlines through block b;
oproj m-tiles trickle at most one per key tile. A 60-matmul warmup holds the
PE busy through the input DMA wait so the HAM clock gate opens (2.4GHz)
before the first projection.
"""

import numpy as np
import ml_dtypes
from contextlib import ExitStack

import concourse.bass as bass
import concourse.tile as tile
from concourse import bacc, mybir
from concourse.bass_utils import run_bass_kernel_spmd

P = 128
S = 4096
D = 1024
DH = 64
N_CORES = 8
SCALE = 1.0 / 8.0  # 1/sqrt(64)
NQ = 512           # query block (matmul free dim)
KT = 128           # key tile (contraction partitions)
NQB = S // NQ      # 8 query blocks
NKT = S // KT      # 32 key tiles
KO = D // P        # 8 contraction subtiles over the model dim

BF16 = mybir.dt.bfloat16
F32 = mybir.dt.float32
EXP = mybir.ActivationFunctionType.Exp
ADD = mybir.AluOpType.add


def _emit(tc, xT, wqT, wkT, wvT, woT, bqk, bvrep, masks, outT, dbg=None):
    nc = tc.nc
    with ExitStack() as ctx:
        from collections import deque
        from concourse.masks import make_identity

        const = ctx.enter_context(tc.tile_pool(name="const", bufs=1))

        # weights/constants first: the first projections need wq + x chunk 0,
        # so nothing small may queue behind the 8.4MB xT transfer. All inputs
        # are host-prepacked to partition-contiguous layouts (128 descriptors
        # per transfer instead of 1024).
        wq_sb = const.tile([P, KO, P], BF16)
        nc.sync.dma_start(wq_sb, wqT)
        bqk_sb = const.tile([P, 3], F32)
        nc.sync.dma_start(bqk_sb, bqk.rearrange("b p -> p b"))
        xT_sb = const.tile([P, NQB, KO, NQ], BF16)
        nc.sync.dma_start(xT_sb[:, 0], xT[:, 0])  # q-proj of block 0 can start
        wk_sb = const.tile([P, KO, P], BF16)
        nc.sync.dma_start(wk_sb, wkT)
        wv_sb = const.tile([P, KO, P], BF16)
        nc.sync.dma_start(wv_sb, wvT)
        masks_sb = const.tile([P, KT], BF16)
        nc.sync.dma_start(masks_sb, masks)
        bv_sb = const.tile([P, P], F32)
        nc.sync.dma_start(bv_sb, bvrep)
        nc.sync.dma_start(xT_sb[:, 1], xT[:, 1])
        wo_sb = const.tile([P, D], BF16)
        nc.sync.dma_start(wo_sb, woT)
        for n in range(2, NQB):  # chunked so projections can start early
            nc.sync.dma_start(xT_sb[:, n], xT[:, n])

        qT_sb = const.tile([P, S], BF16)
        kT_sb = const.tile([P, S], BF16)
        v_sb = const.tile([P, S // P, 130], BF16)
        attnT_sb = const.tile([P, S], BF16)
        nc.vector.memset(v_sb, 1.0)  # presets the two ones-columns

        ident = const.tile([P, P], BF16)
        make_identity(nc, ident)

        # Warm the PE clock (HAM) with throwaway matmuls while the input DMAs
        # land. The HAM needs ~3.4us of *sustained* PE activity to unthrottle
        # (cold MMs run at 1.2GHz, so 60 x N=128 ~ 6.4us) — and the burst must
        # also bridge the DMA wait so the first projections start warm.
        with tc.tile_pool(name="warm_psum", bufs=1, space="PSUM") as wpool:
            wt = wpool.tile([P, P], F32)
            for _ in range(60):
                nc.tensor.matmul(wt, lhsT=ident, rhs=ident, start=True, stop=True)

        # PSUM budget (8 banks): spool 4 (two [128,2,512] score slabs),
        # vpool 2 (pv0/pv1 accumulators), ppool 1 (proj accum / V transpose),
        # opool 1 (output projection).
        spool = ctx.enter_context(tc.tile_pool(name="score_psum", bufs=2, space="PSUM"))
        vpool = ctx.enter_context(tc.tile_pool(name="pv_psum", bufs=1, space="PSUM"))
        ppool = ctx.enter_context(tc.tile_pool(name="proj_psum", bufs=1, space="PSUM"))
        opool = ctx.enter_context(tc.tile_pool(name="oproj_psum", bufs=1, space="PSUM"))
        work = ctx.enter_context(tc.tile_pool(name="work", bufs=7))
        nwork = ctx.enter_context(tc.tile_pool(name="nwork", bufs=4))
        dpool = ctx.enter_context(tc.tile_pool(name="dscratch", bufs=4, space="DRAM"))

        def proj_chunk(bcol, w_sb, dst, n):
            """Two pacing items of 4 accumulation matmuls each (shared psum)."""
            state = {}

            def emit_lo():
                ps = ppool.tile([P, NQ], F32, tag="ps", name=f"ps_{bcol}_{n}")
                state["ps"] = ps
                for kt in range(KO // 2):
                    nc.tensor.matmul(
                        ps,
                        lhsT=w_sb[:, kt, :],
                        rhs=xT_sb[:, n, kt, :],
                        start=(kt == 0),
                        stop=False,
                    )

            def emit_hi():
                ps = state["ps"]
                for kt in range(KO // 2, KO):
                    nc.tensor.matmul(
                        ps,
                        lhsT=w_sb[:, kt, :],
                        rhs=xT_sb[:, n, kt, :],
                        start=False,
                        stop=(kt == KO - 1),
                    )
                nc.vector.tensor_tensor(
                    dst[:, n * NQ:(n + 1) * NQ],
                    ps,
                    bqk_sb[:, bcol:bcol + 1].to_broadcast([P, NQ]),
                    op=ADD,
                )

            return [emit_lo, emit_hi]

        def v_proj(t):
            """V in natural [seq, dh] layout: stationary operand is the xT
            chunk, so no PE transpose is needed afterwards. One seq-tile of
            128 rows lands as [128, 2, 64] inside v_sb (ones columns preset)."""
            def emit():
                ps = ppool.tile([P, P], F32, tag="ps", name=f"vp_{t}")
                for kt in range(KO):
                    nc.tensor.matmul(
                        ps,
                        lhsT=xT_sb[:, t // 4, kt, (t % 4) * P:(t % 4 + 1) * P],
                        rhs=wv_sb[:, kt, :],
                        start=(kt == 0),
                        stop=(kt == KO - 1),
                    )
                nc.vector.tensor_tensor(
                    v_sb[:, t, :].rearrange("p (h x) -> p h x", x=65)[:, :, 0:DH],
                    ps.rearrange("p (h x) -> p h x", x=DH),
                    bv_sb.rearrange("p (h x) -> p h x", x=DH),
                    op=ADD,
                )
            return emit

        def proj_ops(nb):
            ops = []
            ops += proj_chunk(0, wq_sb, qT_sb, nb)
            ops += proj_chunk(1, wk_sb, kT_sb, nb)
            ops += [v_proj(t) for t in range(4 * nb, 4 * nb + 4)]
            return ops

        def oproj_mtile(b, m, tail=False):
            def emit():
                qsl = slice(b * NQ, (b + 1) * NQ)
                # in the drain after the last block, alternate PSUM banks and
                # cast engines so consecutive m-tiles don't serialize on the
                # single oproj bank / the DVE
                use_p = tail and (m % 2 == 1)
                pool, tag = (ppool, "ps") if use_p else (opool, "po")
                po = pool.tile([P, NQ], F32, tag=tag, name=f"po_{b}_{m}")
                nc.tensor.matmul(
                    po,
                    lhsT=wo_sb[:, m * P:(m + 1) * P],
                    rhs=attnT_sb[:, qsl],
                    start=True,
                    stop=True,
                )
                ot = work.tile([P, NQ], BF16, tag="ot", name=f"ot_{b}_{m}")
                if use_p:
                    nc.scalar.copy(ot, po)
                else:
                    nc.vector.tensor_copy(ot, po)
                nc.sync.dma_start(
                    outT.rearrange("(mo p) n -> p mo n", p=P)[:, m, qsl], ot
                )
            return emit

        # block 0's q/k projections up front; its v tiles are only needed by
        # the PV matmuls and would delay the first scores/EXP, so they go
        # into block 0's paced queue instead.
        for op in proj_chunk(0, wq_sb, qT_sb, 0) + proj_chunk(1, wk_sb, kT_sb, 0):
            op()

        ones1 = const.tile([1, DH], F32)
        nc.vector.memset(ones1, 1.0)

        def make_norm(b, pvs, tail=False):
            """Normalize block b's PV accumulators into attnT. Heads are
            interleaved so the DVE works on head 1 while head 0's reciprocal
            broadcast makes its DRAM round-trip. In the tail (last block) the
            broadcast is a K=1 PE matmul instead — the PE is idle there and
            it avoids ~2us of DMA latency on the critical path."""
            def norm():
                qsl = slice(b * NQ, (b + 1) * NQ)
                pvS_t, rb_t = [], []
                for h in (0, 1):
                    if tail:
                        # denominator row first: the reciprocal + broadcast
                        # then overlap the bulk PSUM->SBUF copy
                        rcp0 = nwork.tile([1, NQ], F32, tag=f"rcp0{h}")
                        nc.vector.tensor_copy(rcp0, pvs[h][DH:DH + 1, :])
                        nc.vector.reciprocal_approx_fast(rcp0, rcp0)
                        pvS = nwork.tile([DH + 1, NQ], F32, tag=f"pvS{h}")
                        nc.vector.tensor_copy(pvS, pvs[h])
                    else:
                        # PSUM-freeing copy first: the next block's first PV
                        # matmul reuses this bank and must not wait
                        pvS = nwork.tile([DH + 1, NQ], F32, tag=f"pvS{h}")
                        nc.vector.tensor_copy(pvS, pvs[h])
                        rcp0 = nwork.tile([1, NQ], F32, tag=f"rcp0{h}")
                        nc.vector.tensor_copy(rcp0, pvS[DH:DH + 1, :])
                        nc.vector.reciprocal_approx_fast(rcp0, rcp0)
                    if tail:
                        rb = vpool.tile([DH + 1, NQ], F32, tag=f"pv{h}",
                                        name=f"rb_{b}_{h}")
                        nc.tensor.matmul(rb[0:DH, :], lhsT=ones1, rhs=rcp0,
                                         start=True, stop=True)
                    else:
                        scr = dpool.tile([NQ], F32, tag=f"scr{h}")
                        nc.sync.dma_start(scr, rcp0)
                        rb = nwork.tile([DH, NQ], F32, tag=f"rb{h}")
                        nc.sync.dma_start(rb, scr[None, :].to_broadcast([DH, NQ]))
                    pvS_t.append(pvS)
                    rb_t.append(rb)
                for h in (0, 1):
                    nc.vector.tensor_mul(attnT_sb[h * DH:(h + 1) * DH, qsl],
                                         pvS_t[h][0:DH, :], rb_t[h][0:DH, :])
                    if dbg is not None and b == 1:
                        nc.sync.dma_start(dbg[f"pv{h}"], pvS_t[h])
            return norm

        soft_q = deque()  # oproj items: no hard deadline, trickled 1 per kt
        prev = None       # (emit_pv, state): PV runs one k-tile behind, and
                          # rolls across block boundaries to keep the PE fed
        norm_fn = None    # previous block's normalization, emitted after the
                          # next block's first scores/exp so ACT never waits
        for b in range(NQB):
            nk = 4 * (b + 1)  # causal: only key tiles up to the diagonal
            # q/k/v projections of block b+1 must finish during block b; give
            # each item an even deadline so there is no burst at the boundary.
            hard_q = deque(proj_ops(b + 1)) if b + 1 < NQB else deque()
            if b == 0:
                hard_q.extendleft(reversed([v_proj(t) for t in range(4)]))
            nhard = len(hard_q)
            if b > 0:
                soft_q.extend(oproj_mtile(b - 1, m) for m in range(D // P))
            pvs = [
                vpool.tile([DH + 1, NQ], F32, tag=f"pv{h}", name=f"pv{h}_{b}")
                for h in (0, 1)
            ]

            def emit_pv(st, pvs=pvs, nk=nk):
                pT, kt, q0, nq = st
                for h in (0, 1):
                    nc.tensor.matmul(
                        pvs[h][:, q0:],
                        lhsT=v_sb[:, kt, h * 65:(h + 1) * 65],
                        rhs=pT[:, h, :nq],
                        start=(kt == 0),
                        stop=(kt == nk - 1),
                    )

            for kt in range(nk):
                j = kt - 4 * b  # >= 0 on causal-diagonal key tiles
                # on diagonal tiles only queries >= 128j can attend this tile
                q0 = max(0, j) * KT
                nq = NQ - q0
                qs0 = b * NQ + q0
                # scores go first on the PE queue at every tile: EXP is the
                # pacer and only waits on scores; the previous PV (gated by
                # the diagonal mask) and filler items run in the remainder.
                slab = spool.tile([P, 2, NQ], F32, tag="slab")
                for h in (0, 1):
                    nc.tensor.matmul(
                        slab[:, h, :nq],
                        lhsT=kT_sb[h * DH:(h + 1) * DH, kt * KT:(kt + 1) * KT],
                        rhs=qT_sb[h * DH:(h + 1) * DH, qs0:qs0 + nq],
                        start=True,
                        stop=True,
                    )
                if prev is not None:
                    prev[0](prev[1])
                    prev = None
                pT = work.tile([P, 2, NQ], BF16, tag="pT")
                nc.scalar.activation(pT[:, :, :nq], slab[:, :, :nq], EXP, scale=SCALE)
                if j >= 0:
                    # causal mask: within a diagonal tile only the first 128
                    # queries (qq < 128 relative to q0) form a triangle with
                    # the 128 keys; later queries see the whole tile.
                    nc.vector.tensor_mul(
                        pT[:, :, 0:KT],
                        pT[:, :, 0:KT],
                        masks_sb.rearrange("p (o q) -> p o q", o=1).to_broadcast(
                            [P, 2, KT]
                        ),
                    )
                if norm_fn is not None:
                    norm_fn()
                    norm_fn = None
                prev = (emit_pv, (pT, kt, q0, nq))
                # hard items: evenly spread, finishing ~2 tiles before the
                # boundary so the last bias-add isn't racing the next block's
                # first scores on the DVE
                done_hard = nhard - len(hard_q)
                due = -(-nhard * (kt + 1) // max(1, nk - 2))
                while hard_q and done_hard < due:
                    hard_q.popleft()()
                    done_hard += 1
                # at most one oproj per key tile (single-buffered PSUM bank),
                # and only once the previous block's attnT normalization has
                # had time to finish — else its first matmul blocks the PE FIFO
                if soft_q and kt >= 4:
                    soft_q.popleft()()
            while hard_q:
                hard_q.popleft()()
            norm_fn = make_norm(b, pvs, tail=(b == NQB - 1))
        prev[0](prev[1])
        norm_fn()
        while soft_q:
            soft_q.popleft()()
        for m in range(D // P):
            oproj_mtile(NQB - 1, m, tail=True)()
        if dbg is not None:
            nc.sync.dma_start(dbg["qT"], qT_sb)
            nc.sync.dma_start(dbg["kT"], kT_sb)
            nc.sync.dma_start(dbg["v"], v_sb)
            nc.sync.dma_start(dbg["attnT"], attnT_sb)


def build(debug_out=False):
    nc = bacc.Bacc(
        "TRN2",
        target_bir_lowering=False,
        debug=False,
        enable_asserts=False,
    )
    xT = nc.dram_tensor("xT", [P, NQB, KO, NQ], BF16, kind="ExternalInput").ap()
    wqT = nc.dram_tensor("wqT", [P, KO, P], BF16, kind="ExternalInput").ap()
    wkT = nc.dram_tensor("wkT", [P, KO, P], BF16, kind="ExternalInput").ap()
    wvT = nc.dram_tensor("wvT", [P, KO, P], BF16, kind="ExternalInput").ap()
    woT = nc.dram_tensor("woT", [P, D], BF16, kind="ExternalInput").ap()
    bqk = nc.dram_tensor("bqk", [3, P], F32, kind="ExternalInput").ap()
    bvrep = nc.dram_tensor("bvrep", [P, P], F32, kind="ExternalInput").ap()
    masks = nc.dram_tensor("masks", [P, KT], BF16, kind="ExternalInput").ap()
    outT = nc.dram_tensor("outT", [D, S], BF16, kind="ExternalOutput").ap()
    dbg = None
    if debug_out:
        dbg = {
            "qT": nc.dram_tensor("dbg_qT", [P, S], BF16, kind="ExternalOutput").ap(),
            "kT": nc.dram_tensor("dbg_kT", [P, S], BF16, kind="ExternalOutput").ap(),
            "v": nc.dram_tensor("dbg_v", [P, S // P, 130], BF16, kind="ExternalOutput").ap(),
            "attnT": nc.dram_tensor("dbg_attnT", [P, S], BF16, kind="ExternalOutput").ap(),
            "pv0": nc.dram_tensor("dbg_pv0", [DH + 1, NQ], F32, kind="ExternalOutput").ap(),
            "pv1": nc.dram_tensor("dbg_pv1", [DH + 1, NQ], F32, kind="ExternalOutput").ap(),
        }

    with tile.TileContext(nc) as tc:
        _emit(tc, xT, wqT, wkT, wvT, woT, bqk, bvrep, masks, outT, dbg=dbg)
    nc.compile()
    return nc


def _make_masks():
    k = np.arange(P)[:, None]
    q = np.arange(KT)[None, :]
    return (k <= q).astype(ml_dtypes.bfloat16)


_STATE = {}


def _prep_inputs(x, Wq, bq, Wk, bk, Wv, bv, Wo, bo):
    bf = ml_dtypes.bfloat16
    x2 = np.asarray(x, np.float32).reshape(NQB, NQ, KO, P)
    # xT[p, nb, ko, nq] = x[nb*NQ+nq, ko*P+p]: partition-contiguous chunks
    xT = np.ascontiguousarray(x2.transpose(3, 0, 2, 1)).astype(bf)
    masks = _make_masks()
    Wq = np.asarray(Wq, np.float32)
    Wk = np.asarray(Wk, np.float32)
    Wv = np.asarray(Wv, np.float32)
    Wo = np.asarray(Wo, np.float32)
    bq = np.asarray(bq, np.float32)
    bk = np.asarray(bk, np.float32)
    bv = np.asarray(bv, np.float32)

    def wpack(w_rows):  # [128(m), D] -> [p, ko, m]
        return np.ascontiguousarray(
            w_rows.reshape(P, KO, P).transpose(2, 1, 0)
        ).astype(bf)

    in_maps = []
    for c in range(N_CORES):
        r = slice(c * P, (c + 1) * P)
        in_maps.append({
            "xT": xT,
            "wqT": wpack(Wq[r]),
            "wkT": wpack(Wk[r]),
            "wvT": wpack(Wv[r]),
            "woT": np.ascontiguousarray(Wo[:, r].T).astype(bf),
            "bqk": np.stack([bq[r], bk[r], bv[r]]),
            "bvrep": np.tile(bv[r][None, :], (P, 1)).astype(np.float32),
            "masks": masks,
        })
    return in_maps


def kernel(x, Wq, bq, Wk, bk, Wv, bv, Wo, bo):
    if "nc" not in _STATE:
        _STATE["nc"] = build()
    nc = _STATE["nc"]
    in_maps = _prep_inputs(x, Wq, bq, Wk, bk, Wv, bv, Wo, bo)
    res = run_bass_kernel_spmd(nc, in_maps, core_ids=list(range(N_CORES)))
    total = res.results[0]["outT"].astype(np.float32, copy=True)
    for c in range(1, N_CORES):
        total += res.results[c]["outT"]
    out = total.T + np.asarray(bo, np.float32)[None, :]
    return np.ascontiguousarray(out, dtype=np.float32).reshape(1, S, D)

